# revision 17
# baseline (speedup 1.0000x reference)
"""Trainium2 Bass kernel for nn_CrossModalAttentionBlock (GQA attention + top-2 MoE).

Two SPMD launches over 8 cores:

  L1 "attn" (token-parallel): core c = (batch b=c//2, half=c%2) owns 512 query
    tokens; the host rotates each core's batch sequence so its own half comes
    first. LN1 is folded into the consumers (h = x*a + c per token), so the
    heavy projections run directly on raw x with a rank-1 (-mu * W^T g) matmul
    accumulated into the same psum and a single a-scale on the way out of
    PSUM. Q/K/V/O projections and attn@V run as fp8e4 DoubleRow matmuls (two
    contraction rows per cycle; weights pre-scaled x64 on the host so w~0.02
    stays in the fp8 normal range, the 1/64 folded into the psum post-scale;
    exp writes its fp8 probs directly into the DoubleRow pair slices).
    Scores stay bf16 (two heads packed per PE pass); the gate MLP stays fp32r
    so routing margins are tight.
  Host: top-2 routing mirroring the reference; tokens whose 2nd/3rd gate
    margin is inside the device error envelope are recomputed exactly.
  L2 "moe" (expert-parallel): hidden layer gelu(X@w1) in fp8 DoubleRow,
    out layer @w2 in bf16, scaled by the renormalized gate weight, over
    tokens routed per slot (padded to uniform per-core capacities).
  Host: scatter-add + final residual.

All tensors ship in exactly the layout the engines consume: weights as
[part, ..., pair, 2, cols] DoubleRow stationary blocks, activations as
[part, pair, 2, cols] pair tiles, one large contiguous DMA per tensor,
ordered so the first consumer's bytes land first."""

import numpy as np

import concourse.bass as bass
import concourse.mybir as mybir
import concourse.tile as tile
from concourse import bacc
from concourse.bass_utils import run_bass_kernel_spmd

AF = mybir.ActivationFunctionType
ALU = mybir.AluOpType
FP32 = mybir.dt.float32
FP32R = mybir.dt.float32r
BF16 = mybir.dt.bfloat16
F8 = mybir.dt.float8e4
DR = mybir.MatmulPerfMode.DoubleRow
BF16_NP = mybir.dt.np(BF16)
F8_NP = mybir.dt.np(F8)

B, S, D = 4, 1024, 1024
H, G = 16, 8
HD = D // H              # 64
E, TOPK, ED = 8, 2, 2 * D
GH = D // 2              # 512
EPS = 1e-5
P = 128
NCORES = 8
SQ = S // 2              # 512 query tokens per core
T = B * S
DC = D // P              # 8 feature chunks
NJ = DC // 2             # 4 DoubleRow k-chunk pairs over D
EC = ED // P             # 16 hidden chunks
SCALE = HD ** -0.5
WS = 64.0                # fp8 weight scale
US = 64.0                # fp8 attention-output scale

LO = [0, 1, 4, 5, 8, 9, 12, 13]
HI = [2, 3, 6, 7, 10, 11, 14, 15]
SLOT_HEAD = [h for p in range(8) for h in (LO[p], HI[p])]

# Routing margin below which the host recomputes gate logits exactly.
SUS_MARGIN = 2.5e-3


# ------------------------------------------------------------- host helpers --

def _pair_w(w, scale=WS):
    """[K, M] weight -> [128, M/128, K/256, 2, 128] fp8 DoubleRow blocks."""
    K, M = w.shape
    a = (np.asarray(w, np.float32) * scale).astype(F8_NP)
    a = a.reshape(K // 256, 2, P, M // P, P).transpose(2, 3, 0, 1, 4)
    return np.ascontiguousarray(a)


def _softmax_np(x, axis=-1):
    m = x.max(axis=axis, keepdims=True)
    e = np.exp(x - m)
    return e / e.sum(axis=axis, keepdims=True)


# ------------------------------------------------------------------ L1 attn --

def build_attn():
    nc = bacc.Bacc("TRN2", target_bir_lowering=False, debug=False, num_devices=NCORES)

    xp_d = nc.dram_tensor("xp", [P, NJ, 2, S], F8, kind="ExternalInput").ap()
    xq_d = nc.dram_tensor("xq", [P, DC, SQ], BF16, kind="ExternalInput").ap()
    wq_d = nc.dram_tensor("wq_p", [P, 4, NJ, 2, P], F8, kind="ExternalInput").ap()
    wk_d = nc.dram_tensor("wk_p", [P, DC, NJ, 2, P], F8, kind="ExternalInput").ap()
    wv_d = nc.dram_tensor("wv_p", [P, 2, NJ, 2, 512], F8, kind="ExternalInput").ap()
    wo_d = nc.dram_tensor("wo_p", [P, DC, NJ, 2, P], F8, kind="ExternalInput").ap()
    gw1_d = nc.dram_tensor("gw1_p", [P, 4, DC, P], FP32R, kind="ExternalInput").ap()
    gw2_d = nc.dram_tensor("gw2_p", [P, 4, E], FP32R, kind="ExternalInput").ap()
    # rank-1 row tables bf16 (x WS): qg1[512], kg1[1024], vg1[1024]
    r1_d = nc.dram_tensor("r1t", [1, 512 + D + D], BF16, kind="ExternalInput").ap()
    gg1_d = nc.dram_tensor("gg1", [1, 512], FP32R, kind="ExternalInput").ap()
    pcs_d = nc.dram_tensor("pcs", [P, 2 * DC], FP32, kind="ExternalInput").ap()
    gb2_d = nc.dram_tensor("gb2", [E, 1], FP32, kind="ExternalInput").ap()

    x1T_d = nc.dram_tensor("x1T", [P, DC, SQ], FP32, kind="ExternalOutput").ap()
    h2T_d = nc.dram_tensor("h2T", [P, DC, SQ], F8, kind="ExternalOutput").ap()
    glogT_d = nc.dram_tensor("glogT", [E, SQ], FP32, kind="ExternalOutput").ap()

    with tile.TileContext(nc) as tc:
        import contextlib
        ctx = contextlib.ExitStack()
        with ctx:
            const = ctx.enter_context(tc.tile_pool(name="const", bufs=1))
            rows = ctx.enter_context(tc.tile_pool(name="rows", bufs=2))
            bcast = ctx.enter_context(tc.tile_pool(name="bcast", bufs=2))
            tmp_f = ctx.enter_context(tc.tile_pool(name="tmpf", bufs=2))
            qt_pool = ctx.enter_context(tc.tile_pool(name="qt", bufs=4))
            ut_pool = ctx.enter_context(tc.tile_pool(name="ut", bufs=NJ))
            xin = ctx.enter_context(tc.tile_pool(name="xin", bufs=1))
            wts = ctx.enter_context(tc.tile_pool(name="wts", bufs=1))
            ps_main = ctx.enter_context(tc.tile_pool(name="psm", bufs=2, space="PSUM"))
            ps_sc = ctx.enter_context(tc.tile_pool(name="pssc", bufs=2, space="PSUM"))
            ps_att = ctx.enter_context(tc.tile_pool(name="psat", bufs=2, space="PSUM"))
            ps_nrm = ctx.enter_context(tc.tile_pool(name="psnr", bufs=2, space="PSUM"))

            # ---- inputs: earliest consumer's bytes first --------------------
            xp = xin.tile([P, NJ, 2, S], F8, tag="xp", name="xp")
            nc.sync.dma_start(xp[:, :, :, 0:512], xp_d[:, :, :, 0:512])
            nc.sync.dma_start(xp[:, :, :, 512:1024], xp_d[:, :, :, 512:1024])
            wk8 = wts.tile([P, DC, NJ, 2, P], F8, tag="wk8", name="wk8")
            nc.scalar.dma_start(wk8[:], wk_d[:])
            wq8 = wts.tile([P, 4, NJ, 2, P], F8, tag="wq8", name="wq8")
            nc.scalar.dma_start(wq8[:], wq_d[:])
            wv8 = wts.tile([P, 2, NJ, 2, 512], F8, tag="wv8", name="wv8")
            wo8 = wts.tile([P, DC, NJ, 2, P], F8, tag="wo8", name="wo8")
            gwb = wts.tile([P, 4, DC, P], FP32R, tag="gw1", name="gw1")
            gw2b = wts.tile([P, 4, E], FP32R, tag="gw2", name="gw2")
            xq = xin.tile([P, DC, SQ], BF16, tag="xq", name="xq")

            # ---- constants -------------------------------------------------
            ones_f = const.tile([P, 1], FP32)
            nc.vector.memset(ones_f[:], 1.0)
            ones_r = const.tile([P, 1], FP32R)
            nc.scalar.copy(ones_r[:], ones_f[:])
            ones_row_f = const.tile([1, P], FP32)
            nc.vector.memset(ones_row_f[:], 1.0)
            ones_row = const.tile([1, P], FP32R)
            nc.scalar.copy(ones_row[:], ones_row_f[:])
            ones_sq_f = const.tile([P, 64], FP32)
            nc.vector.memset(ones_sq_f[:], 1.0)
            ones_sq = const.tile([P, 64], FP32R)
            nc.scalar.copy(ones_sq[:], ones_sq_f[:])
            ones16 = const.tile([1, 1], BF16)
            nc.vector.memset(ones16[:], 1.0)
            ones8_t = const.tile([P, 2, 16], F8)
            nc.vector.memset(ones8_t[:], 1.0)
            ones8 = ones8_t[:, :, 0:1]      # pair-axis step 16 (ISA: step%16==0)
            c4096 = const.tile([P, 1], FP32)
            nc.vector.memset(c4096[:], 1.0 / (WS * US))
            r1t = const.tile([1, 512 + D + D], BF16, tag="r1t", name="r1t")
            nc.sync.dma_start(r1t[:], r1_d[:])
            qg1 = r1t[:, 0:512]
            kg1 = r1t[:, 512:512 + D]
            vg1 = r1t[:, 512 + D:512 + 2 * D]
            Gg1 = const.tile([1, 512], FP32R, tag="gg1", name="gg1")
            nc.sync.dma_start(Gg1[:], gg1_d[:])
            Gg1 = Gg1[:, :]
            pcs = const.tile([P, 2 * DC], FP32, tag="pcs", name="pcs")
            nc.sync.dma_start(pcs[:], pcs_d[:])
            g2_pc = pcs[:, 0:DC]
            b2_pc = pcs[:, DC:2 * DC]
            gb2_pc = const.tile([E, 1], FP32)
            nc.sync.dma_start(gb2_pc[:], gb2_d[:])
            eps_b = const.tile([1, 1], FP32)
            nc.vector.memset(eps_b[:], float(EPS))

            # PE warm-up while the xp DMA lands
            warm = const.tile([P, P], BF16)
            nc.vector.memset(warm[:], 0.0)
            psw = ps_sc.tile([P, P], FP32, tag="ps_s", name="psw")
            for i in range(30):
                nc.tensor.matmul(psw[:], warm[:], warm[:], start=True, stop=True)

            # ---- LN1 stats (window n of 512 tokens) -------------------------
            att_ctx = contextlib.ExitStack()
            sq_pool = att_ctx.enter_context(tc.tile_pool(name="sqp", bufs=1))
            sq8 = sq_pool.tile([P, NJ, 2, S], F8, tag="sq8", name="sq8")

            stats = {}

            def bcast_rows(psum_pool, tag, row, w, name):
                """[1, w] row -> [128, w] sbuf via a K=1 PE matmul + copy."""
                psb = psum_pool.tile([P, w], FP32, tag=tag, name=f"psb_{name}")
                nc.tensor.matmul(psb[:], ones_row[:], row[:],
                                 start=True, stop=True)
                out = bcast.tile([P, w], FP32, tag="a_b", name=f"bc_{name}")
                nc.vector.tensor_copy(out[:], psb[:])
                return out

            def stats_win1(n):
                w = 512
                sl = slice(n * 512, (n + 1) * 512)
                for j in range(NJ):
                    for i in range(2):
                        nc.vector.tensor_tensor(sq8[:, j, i, sl], xp[:, j, i, sl],
                                                xp[:, j, i, sl], ALU.mult)
                psx = ps_main.tile([1, 512], FP32, tag="ps", name=f"psx{n}")
                psq = ps_main.tile([1, 512], FP32, tag="ps", name=f"psq{n}")
                for j in range(NJ):
                    nc.tensor.matmul(psx[:], ones8, xp[:, j, :, sl],
                                     start=(j == 0), stop=(j == NJ - 1), perf_mode=DR)
                for j in range(NJ):
                    nc.tensor.matmul(psq[:], ones8, sq8[:, j, :, sl],
                                     start=(j == 0), stop=(j == NJ - 1), perf_mode=DR)
                mu_row = rows.tile([1, w], FP32, tag="mu", name=f"mu{n}")
                var_row = rows.tile([1, w], FP32, tag="var", name=f"var{n}")
                t_row = rows.tile([1, w], FP32, tag="t", name=f"t{n}")
                nc.vector.tensor_scalar_mul(mu_row[:], psx[:1, :], 1.0 / D)
                nc.vector.tensor_tensor(t_row[:], mu_row[:], mu_row[:], ALU.mult)
                nc.vector.scalar_tensor_tensor(var_row[:], psq[:1, :], 1.0 / D,
                                               t_row[:], ALU.mult, ALU.subtract)
                sd_row = rows.tile([1, w], FP32, tag="t", name=f"sd{n}")
                nc.scalar.activation(sd_row[:], var_row[:], AF.Sqrt, bias=eps_b[:])
                a_row = rows.tile([1, w], FP32, tag="var", name=f"a{n}")
                nc.vector.reciprocal_approx_fast(out=a_row[:], in_=sd_row[:])
                aS_row = rows.tile([1, w], FP32R, tag="as", name=f"as{n}")
                nc.vector.tensor_scalar_mul(aS_row[:], a_row[:], 1.0 / WS)
                nmu_row = rows.tile([1, w], BF16, tag="mu2", name=f"nmu{n}")
                nc.vector.tensor_scalar_mul(nmu_row[:], mu_row[:], -1.0)
                aS16_row = rows.tile([1, w], BF16, tag="as16", name=f"as16_{n}")
                nc.vector.tensor_scalar_mul(aS16_row[:], a_row[:], 1.0 / WS)
                a_bS = bcast_rows(ps_main, "ps", aS_row, w, f"a{n}")
                stats[n] = (a_bS, nmu_row, aS16_row)

            att_ctx2 = contextlib.ExitStack()
            kt_pool = att_ctx2.enter_context(tc.tile_pool(name="kt", bufs=DC))
            va_pool = att_ctx2.enter_context(tc.tile_pool(name="va", bufs=NJ))
            ex_pool = att_ctx2.enter_context(tc.tile_pool(name="ex", bufs=14))
            nrm = att_ctx2.enter_context(tc.tile_pool(name="nrm", bufs=2))

            QT = [qt_pool.tile([P, SQ], BF16, tag="QT", name=f"QT{i}") for i in range(4)]
            KTH = [kt_pool.tile([P, S], BF16, tag="KTH", name=f"KTH{i}") for i in range(DC)]
            # V pairs: [key-part, kc-pair-slice, head-slot, 64 v | 1 ones]
            V_p = [va_pool.tile([P, 2, 16, 65], F8, tag="V_p", name=f"V_p{j}")
                   for j in range(NJ)]
            UTp = [ut_pool.tile([P, 2, SQ], F8, tag="UTp", name=f"UTp{i}")
                   for i in range(NJ)]
            for j in range(NJ):
                nc.vector.memset(V_p[j][:, :, :, 64:65], 1.0)
            acol = const.tile([P, DC], FP32, tag="acol", name="acol")

            def q_proj(ms):
                a_bS, nmu_row = stats[0][0], stats[0][1]
                for m in ms:
                    psq = ps_main.tile([P, 512], FP32, tag="ps", name=f"psQ{m}")
                    for j in range(NJ):
                        nc.tensor.matmul(psq[:], wq8[:, m, j], xp[:, j, :, 0:SQ],
                                         start=(j == 0), stop=False, perf_mode=DR)
                    nc.tensor.matmul(psq[:], qg1[:, m * P:(m + 1) * P], nmu_row[:],
                                     start=False, stop=True, skip_group_check=True)
                    nc.vector.tensor_tensor(QT[m][:], psq[:], a_bS[:], ALU.mult)

            def k_proj(p, n):
                a_bS, nmu_row = stats[n][0], stats[n][1]
                sl = slice(n * 512, (n + 1) * 512)
                psk = ps_main.tile([P, 512], FP32, tag="ps", name=f"psK{p}_{n}")
                for j in range(NJ):
                    nc.tensor.matmul(psk[:], wk8[:, p, j], xp[:, j, :, sl],
                                     start=(j == 0), stop=False, perf_mode=DR)
                nc.tensor.matmul(psk[:], kg1[:, p * P:(p + 1) * P], nmu_row[:],
                                 start=False, stop=True, skip_group_check=True)
                nc.vector.tensor_tensor(KTH[p][:, sl], psk[:], a_bS[:], ALU.mult)

            def v_proj(n, scs):
                for sc in scs:
                    w = sc // 4     # token window of this block
                    nmu_row = stats[w][1]
                    psv = ps_main.tile([P, 512], FP32, tag="ps", name=f"psV{n}_{sc}")
                    for j in range(NJ):
                        nc.tensor.matmul(psv[:], xp[:, j, :, sc * P:(sc + 1) * P],
                                         wv8[:, n, j], start=(j == 0), stop=False,
                                         perf_mode=DR)
                    nc.tensor.matmul(psv[:],
                                     nmu_row[:, (sc % 4) * P:(sc % 4 + 1) * P],
                                     vg1[:, n * 512:(n + 1) * 512],
                                     start=False, stop=True, skip_group_check=True)
                    nc.vector.tensor_scalar(
                        V_p[sc // 2][:, sc % 2, n * 8:(n + 1) * 8, 0:64],
                        psv.rearrange("p (h d) -> p h d", d=64),
                        acol[:, sc:sc + 1], None, ALU.mult)

            def sc_half(p, hi, js=range(NJ)):
                off = hi * 64
                slot = 2 * p + hi
                g = SLOT_HEAD[slot] // 2
                mq, qoff = g // 2, (g % 2) * 64
                assert qoff == off
                expS = sc_half.exp.setdefault(slot, {})
                for j in js:
                    expS[j] = ex_pool.tile([P, 2, SQ], F8, tag="expS",
                                           name=f"expS{slot}_{j}")
                    for i in range(2):
                        kc = 2 * j + i
                        pss = ps_sc.tile([P, 512], FP32, tag="ps_s",
                                         name=f"s{slot}_{kc}")
                        nc.tensor.matmul(pss[:],
                                         KTH[p][off:off + 64, kc * P:(kc + 1) * P],
                                         QT[mq][qoff:qoff + 64, :],
                                         start=True, stop=True)
                        nc.scalar.activation(expS[j][:, i, :], pss[:], AF.Exp,
                                             scale=SCALE)
            sc_half.exp = {}

            def av_pair(p):
                psas = []
                for hi in range(2):
                    slot = 2 * p + hi
                    expS = sc_half.exp.pop(slot)
                    psa = ps_att.tile([65, 512], FP32, tag="pa", name=f"a{slot}")
                    for j in range(NJ):
                        nc.tensor.matmul(psa[:], V_p[j][:, :, slot, :], expS[j][:],
                                         start=(j == 0), stop=(j == NJ - 1),
                                         perf_mode=DR)
                    del expS
                    psas.append(psa)
                j, i = p // 2, p % 2
                den_sb = nrm.tile([65, 1024], FP32R, tag="den", name=f"ds{p}")
                recs = []
                for hi in range(2):
                    # den row lives on partition 64; broadcast it down to the
                    # 64 value partitions with a K=1 matmul, then reciprocal.
                    nc.vector.tensor_copy(den_sb[64:65, hi * 512:(hi + 1) * 512],
                                          psas[hi][64:65, :])
                    psb = ps_nrm.tile([64, 512], FP32, tag="psb", name=f"pb{p}_{hi}")
                    nc.tensor.matmul(psb[:], ones_sq[64:65, 0:64],
                                     den_sb[64:65, hi * 512:(hi + 1) * 512],
                                     start=True, stop=True)
                    denb = nrm.tile([64, 512], FP32, tag=f"db{hi}", name=f"db{p}_{hi}")
                    nc.vector.tensor_copy(denb[:], psb[:])
                    rec = nrm.tile([64, 512], FP32, tag=f"rc{hi}", name=f"rc{p}_{hi}")
                    nc.vector.reciprocal_approx_fast(out=rec[:], in_=denb[:])
                    recs.append(rec)
                nc.vector.scalar_tensor_tensor(UTp[j][0:64, i, :], psas[0][0:64, :],
                                               US, recs[0][:], ALU.mult, ALU.mult)
                nb = nrm.tile([64, 512], F8, tag="nb", name=f"nb{p}")
                nc.vector.scalar_tensor_tensor(nb[:], psas[1][0:64, :],
                                               US, recs[1][:], ALU.mult, ALU.mult)
                nc.sync.dma_start(UTp[j][64:128, i, :], nb[:])

            # ---- schedule --------------------------------------------------
            def acols(n):
                for sc in range(n * 4, n * 4 + 4):
                    aS16_row = stats[n][2]
                    ptp = ps_main.tile([P, 1], FP32, tag="ps", name=f"tp{sc}")
                    nc.tensor.matmul(ptp[:], aS16_row[:, (sc % 4) * P:(sc % 4 + 1) * P],
                                     ones16[0:1, :], start=True, stop=True)
                    nc.vector.tensor_copy(acol[:, sc:sc + 1], ptp[:])

            stats_win1(0)
            stats_win1(1)
            nc.scalar.dma_start(wv8[:], wv_d[:])
            acols(0)
            acols(1)
            q_proj([0])
            k_proj(0, 0)
            k_proj(0, 1)
            sc_half(0, 0)
            sc_half(0, 1)
            q_proj([1, 2, 3])
            k_proj(1, 0)
            v_proj(0, range(4))
            k_proj(1, 1)
            v_proj(0, range(4, 8))
            av_pair(0)
            nc.scalar.dma_start(wo8[:], wo_d[:])
            nc.scalar.dma_start(gwb[:], gw1_d[:])
            nc.scalar.dma_start(gw2b[:], gw2_d[:])
            nc.scalar.dma_start(xq[:], xq_d[:])
            for p in range(1, 8):
                sc_half(p, 0)
                sc_half(p, 1)
                # filler between scores and attnV hides the exp latency
                if p == 1:
                    v_proj(1, range(4))
                    k_proj(2, 0)
                    k_proj(2, 1)
                elif p == 2:
                    v_proj(1, range(4, 8))
                    k_proj(3, 0)
                    k_proj(3, 1)
                elif p < 7:
                    k_proj(p + 1, 0)
                    k_proj(p + 1, 1)
                av_pair(p)
            att_ctx2.close()     # free KTH/V_p/expS space for the tail
            att_ctx.close()      # free sq8

            # late pools, in space vacated by the attention working set
            x1_pool = ctx.enter_context(tc.tile_pool(name="x1", bufs=1))
            h2_pool = ctx.enter_context(tc.tile_pool(name="h2", bufs=2))
            gh_pool = ctx.enter_context(tc.tile_pool(name="gh", bufs=4))
            sq2_pool = ctx.enter_context(tc.tile_pool(name="sq2", bufs=2))

            # ---- tail: out-projection + residual, LN2, folded gate ---------
            x1T = x1_pool.tile([P, DC, SQ], FP32R, tag="x1T", name="x1T")
            h2T = h2_pool.tile([P, DC, SQ], F8, tag="h2T", name="h2T")
            GhT = [gh_pool.tile([P, SQ], FP32R, tag="GhT", name=f"GhT{i}")
                   for i in range(4)]
            glog_sb = rows.tile([E, SQ], FP32, tag="glog", name="glog")
            st2 = {}

            def out_proj(w):
                sl = slice(w * 256, (w + 1) * 256)
                for m in range(DC):
                    pso = ps_main.tile([P, 256], FP32, tag="ps", name=f"psO{w}_{m}")
                    for j in range(NJ):
                        nc.tensor.matmul(pso[:], wo8[:, m, j], UTp[j][:, :, sl],
                                         start=(j == 0), stop=(j == NJ - 1),
                                         perf_mode=DR)
                    nc.vector.scalar_tensor_tensor(x1T[:, m, sl], pso[:], c4096[:],
                                                   xq[:, m, sl], ALU.mult, ALU.add)
                nc.sync.dma_start(x1T_d[:, :, sl], x1T.bitcast(FP32)[:, :, sl])

            def stats_win2(w):
                sl = slice(w * 256, (w + 1) * 256)
                ww = 256
                psx = ps_att.tile([1, 256], FP32, tag="pa", name=f"psx2_{w}")
                psq = ps_att.tile([1, 256], FP32, tag="pa", name=f"psq2_{w}")
                for kd in range(DC):
                    sq = sq2_pool.tile([P, 256], FP32R, tag="sqt", name=f"sq2_{w}_{kd}")
                    nc.vector.tensor_tensor(sq[:], x1T[:, kd, sl], x1T[:, kd, sl],
                                            ALU.mult)
                    nc.tensor.matmul(psx[:1, :], ones_r[:], x1T[:, kd, sl],
                                     start=(kd == 0), stop=(kd == DC - 1))
                    nc.tensor.matmul(psq[:1, :], ones_r[:], sq[:],
                                     start=(kd == 0), stop=(kd == DC - 1))
                mu_row = rows.tile([1, ww], FP32, tag="mu", name=f"mu2{w}")
                var_row = rows.tile([1, ww], FP32, tag="var", name=f"var2{w}")
                t_row = rows.tile([1, ww], FP32, tag="t", name=f"t2{w}")
                nc.vector.tensor_scalar_mul(mu_row[:], psx[:1, :], 1.0 / D)
                nc.vector.tensor_tensor(t_row[:], mu_row[:], mu_row[:], ALU.mult)
                nc.vector.scalar_tensor_tensor(var_row[:], psq[:1, :], 1.0 / D,
                                               t_row[:], ALU.mult, ALU.subtract)
                sd_row = rows.tile([1, ww], FP32, tag="t", name=f"sd2{w}")
                nc.scalar.activation(sd_row[:], var_row[:], AF.Sqrt, bias=eps_b[:])
                a_row = rows.tile([1, ww], FP32, tag="var", name=f"a2{w}")
                nc.vector.reciprocal_approx_fast(out=a_row[:], in_=sd_row[:])
                a_rowr = rows.tile([1, ww], FP32R, tag="ar", name=f"a2r{w}")
                nc.vector.tensor_scalar_mul(a_rowr[:], a_row[:], 1.0)
                nmu_row = rows.tile([1, ww], FP32R, tag="mu2", name=f"nmu2{w}")
                nc.vector.tensor_scalar_mul(nmu_row[:], mu_row[:], -1.0)
                c_row = rows.tile([1, ww], FP32R, tag="c", name=f"c2{w}")
                nc.vector.tensor_tensor(c_row[:], nmu_row.bitcast(FP32)[:],
                                        a_row[:], ALU.mult)
                a_b = bcast_rows(ps_nrm, "psb", a_rowr, ww, f"a2_{w}")
                c_b = bcast_rows(ps_nrm, "psb", c_row, ww, f"c2_{w}")
                st2[w] = (a_b, nmu_row, c_b)

            def gate_win(w):
                a2_b, nmu2_row = st2[w][0], st2[w][1]
                sl = slice(w * 256, (w + 1) * 256)
                for m in range(4):
                    psg = ps_sc.tile([P, 256], FP32, tag="ps_s", name=f"psG{w}_{m}")
                    for kd in range(DC):
                        nc.tensor.matmul(psg[:], gwb[:, m, kd, :], x1T[:, kd, sl],
                                         start=(kd == 0), stop=False)
                    nc.tensor.matmul(psg[:], Gg1[:, m * P:(m + 1) * P],
                                     nmu2_row[:], start=False, stop=True)
                    pre = h2_pool.tile([P, 256], FP32, tag="pre", name=f"pre{w}_{m}")
                    nc.vector.tensor_tensor(pre[:], psg[:], a2_b[:], ALU.mult)
                    nc.vector.tensor_scalar_max(GhT[m][:, sl], pre[:], 0.0)
                psl = ps_att.tile([E, 256], FP32, tag="pa", name=f"psl{w}")
                for gt in range(4):
                    nc.tensor.matmul(psl[:], gw2b[:, gt, :], GhT[gt][:, sl],
                                     start=(gt == 0), stop=(gt == 3))
                nc.vector.tensor_scalar(glog_sb[:, sl], psl[:], gb2_pc[:], None,
                                        ALU.add)

            def h2_win(w):
                sl = slice(w * 256, (w + 1) * 256)
                a2_b, c2_b = st2[w][0], st2[w][2]
                for kd in range(DC):
                    t = tmp_f.tile([P, 256], FP32, tag="lnt2", name=f"l2_{w}_{kd}")
                    nc.vector.tensor_tensor(t[:], x1T[:, kd, sl], a2_b[:], ALU.mult)
                    nc.vector.tensor_tensor(t[:], t[:], c2_b[:], ALU.add)
                    nc.vector.tensor_scalar(h2T[:, kd, sl], t[:],
                                            g2_pc[:, kd:kd + 1],
                                            b2_pc[:, kd:kd + 1], ALU.mult, ALU.add)
                nc.sync.dma_start(h2T_d[:, :, sl], h2T[:, :, sl])

            out_proj(0)
            stats_win2(0)
            out_proj(1)
            gate_win(0)
            stats_win2(1)
            h2_win(0)
            gate_win(1)
            h2_win(1)
            nc.sync.dma_start(glogT_d[:], glog_sb[:])

    nc.compile()
    return nc


# ------------------------------------------------------------------- L2 moe --

def _windows(cap):
    ws = [512] * (cap // 512)
    if cap % 512:
        ws.append(cap % 512)
    return ws


def build_ffn(caps):
    """One FFN slot per entry in `caps` (uniform shapes across cores).
    Layer 1 fp8 DoubleRow, layer 2 bf16."""
    nc = bacc.Bacc("TRN2", target_bir_lowering=False, debug=False, num_devices=NCORES)

    ins, outs = [], []
    for si, cap in enumerate(caps):
        ins.append(dict(
            xg=nc.dram_tensor(f"xg{si}", [P, NJ, 2, cap], F8, kind="ExternalInput").ap(),
            w1=nc.dram_tensor(f"w1_{si}", [P, EC, NJ, 2, P], F8,
                              kind="ExternalInput").ap(),
            w2=nc.dram_tensor(f"w2_{si}", [P, DC, EC, P], BF16,
                              kind="ExternalInput").ap(),
            eb1=nc.dram_tensor(f"eb1_{si}", [P, EC], FP32, kind="ExternalInput").ap(),
            eb2=nc.dram_tensor(f"eb2_{si}", [P, DC], FP32, kind="ExternalInput").ap(),
            wt=nc.dram_tensor(f"wt{si}", [1, cap], FP32, kind="ExternalInput").ap(),
        ))
        outs.append(nc.dram_tensor(f"y{si}", [P, DC, cap], BF16,
                                   kind="ExternalOutput").ap())

    with tile.TileContext(nc) as tc:
        import contextlib
        ctx = contextlib.ExitStack()
        with ctx:
            const = ctx.enter_context(tc.tile_pool(name="const", bufs=1))
            xg_pool = ctx.enter_context(tc.tile_pool(name="xg", bufs=1))
            hid_pool = ctx.enter_context(tc.tile_pool(name="hid", bufs=1))
            w_pool = ctx.enter_context(tc.tile_pool(name="wp", bufs=1))
            out_pool = ctx.enter_context(tc.tile_pool(name="out", bufs=1))
            ps = ctx.enter_context(tc.tile_pool(name="ps", bufs=6, space="PSUM"))

            # sync queue: small/early tensors in consumer order; the big w2
            # transfers are issued mid-stream (scalar queue) so they don't
            # starve the layer-1 inputs.
            tls = []
            for si, cap in enumerate(caps):
                io = ins[si]
                xgt = xg_pool.tile([P, NJ, 2, cap], F8, tag=f"xg_{si}",
                                   name=f"xg_{si}")
                w1t = w_pool.tile([P, EC, NJ, 2, P], F8, tag=f"w1_{si}",
                                  name=f"w1_{si}")
                if si == 0:
                    nc.sync.dma_start(xgt[:], io["xg"][:])
                    nc.sync.dma_start(w1t[:], io["w1"][:])
                eb1_pc = const.tile([P, EC], FP32, tag="eb1", name=f"eb1_{si}")
                nc.sync.dma_start(eb1_pc[:], io["eb1"][:])
                eb2_pc = const.tile([P, DC], FP32, tag="eb2", name=f"eb2_{si}")
                nc.sync.dma_start(eb2_pc[:], io["eb2"][:])
                wt_row = const.tile([1, caps[0]], FP32, tag="wtr", name=f"wtr{si}")
                nc.sync.dma_start(wt_row[:1, :cap], io["wt"][:])
                wt_b = const.tile([P, caps[0]], FP32, tag="wtb", name=f"wtb{si}")
                nc.gpsimd.partition_broadcast(wt_b[:, :cap], wt_row[:1, :cap])
                w2t = w_pool.tile([P, DC, EC, P], BF16, tag=f"w2_{si}",
                                  name=f"w2_{si}")
                tls.append((w1t, xgt, w2t, eb1_pc, eb2_pc, wt_b))

            # PE warm-up while the first DMAs land
            warm = const.tile([P, P], BF16)
            nc.vector.memset(warm[:], 0.0)
            psw = ps.tile([P, P], FP32, tag="ph", name="psw")
            for i in range(60):
                nc.tensor.matmul(psw[:], warm[:], warm[:], start=True, stop=True)

            for si, cap in enumerate(caps):
                w1t, xgt, w2t, eb1_pc, eb2_pc, wt_b = tls[si]
                io = ins[si]
                WSl = _windows(cap)
                OFF = [sum(WSl[:i]) for i in range(len(WSl))]

                hidT = hid_pool.tile([P, EC, cap], BF16, tag=f"hidT{si}",
                                     name=f"hidT{si}")
                for wi, w in enumerate(WSl):
                    sl = slice(OFF[wi], OFF[wi] + w)
                    for ec in range(EC):
                        ph = ps.tile([P, w], FP32, tag="ph", name=f"ph{si}_{ec}_{wi}")
                        for j in range(NJ):
                            nc.tensor.matmul(ph[:], w1t[:, ec, j], xgt[:, j, :, sl],
                                             start=(j == 0), stop=(j == NJ - 1),
                                             perf_mode=DR)
                        nc.scalar.activation(hidT[:, ec, sl], ph[:], AF.Gelu,
                                             bias=eb1_pc[:, ec:ec + 1], scale=1.0 / WS)
                        if wi == 0 and si == 0 and ec == 1:
                            # slot-1 layer-1 inputs load while slot-0 computes
                            nc.scalar.dma_start(tls[1][1][:], ins[1]["xg"][:])
                            nc.scalar.dma_start(tls[1][0][:], ins[1]["w1"][:])
                        if wi == 0 and ec == 3:
                            # big w2 load starts once L1 is underway
                            nc.scalar.dma_start(w2t[:], io["w2"][:])

                ostage = out_pool.tile([P, DC, cap], BF16, tag=f"os{si}",
                                       name=f"os{si}")
                for wi, w in enumerate(WSl):
                    sl = slice(OFF[wi], OFF[wi] + w)
                    for m in range(DC):
                        py = ps.tile([P, w], FP32, tag="ph", name=f"py{si}_{m}_{wi}")
                        for et in range(EC):
                            nc.tensor.matmul(py[:], w2t[:, m, et, :], hidT[:, et, sl],
                                             start=(et == 0), stop=(et == EC - 1))
                        nc.vector.scalar_tensor_tensor(ostage[:, m, sl], py[:],
                                                       eb2_pc[:, m:m + 1],
                                                       wt_b[:, sl], ALU.add, ALU.mult)
                nc.sync.dma_start(outs[si][:], ostage[:])

    nc.compile()
    return nc


def _pack_slots(tok_lists, wt_lists):
    """Cut per-expert token lists into at most 8 slot-1 pieces (<= c1) and 8
    slot-2 pieces (<= c2), minimizing the uniform SPMD capacities c1 + c2."""
    loads = [len(t) for t in tok_lists]
    act = [e for e in range(len(loads)) if loads[e] > 0]

    def feas(c1, c2):
        n1 = {e: 0 for e in act}
        n2 = {e: -(-loads[e] // c2) for e in act}
        for _ in range(64):
            if sum(n1.values()) > NCORES:
                return None
            if sum(n2.values()) <= NCORES:
                return n1, n2
            def gain(e):
                rem = loads[e] - n1[e] * c1
                if rem <= 0:
                    return (-1, 0)
                new = -(-max(0, rem - c1) // c2)
                return (n2[e] - new, rem)
            e = max(act, key=gain)
            if gain(e)[0] <= 0:
                return None
            n1[e] += 1
            n2[e] = -(-max(0, loads[e] - n1[e] * c1) // c2)
        return None

    best = None
    for c1 in range(512, 3392, 32):
        if best is not None and best[0] <= c1 + 256:
            break
        for c2 in range(256, c1 + 32, 32):
            if best is not None and c1 + c2 >= best[0]:
                break
            r = feas(c1, c2)
            if r is not None:
                best = (c1 + c2, c1, c2, r[0], r[1])
    _, c1, c2, n1, n2 = best
    s1, s2 = [], []
    for e in act:
        off = 0
        for _ in range(n1[e]):
            sz = min(c1, loads[e] - off)
            s1.append((e, off, sz))
            off += sz
        rem = loads[e] - off
        if rem > 0:
            psz = -(-rem // n2[e])
            for _ in range(n2[e]):
                sz = min(psz, loads[e] - off)
                if sz > 0:
                    s2.append((e, off, sz))
                    off += sz
    assert len(s1) <= NCORES and len(s2) <= NCORES
    assignment = []
    for core in range(NCORES):
        slots = []
        for group in (s1, s2):
            if core < len(group):
                e, off, sz = group[core]
                slots.append((e, tok_lists[e][off:off + sz], wt_lists[e][off:off + sz]))
            else:
                slots.append((0, np.zeros(0, np.int64), np.zeros(0, np.float32)))
        assignment.append(slots)
    return (c1, c2), assignment


# --------------------------------------------------------------- host logic --

_CACHE = {}


def _exact_gate_rows(x, wq, bq, wk, bk, wv, bv, wo, bo, ln1g, ln1b, ln2g, ln2b,
                     gw1, gb1, gw2, gb2, toks):
    """Exact (float64, vectorized) gate logits for the given flat token ids."""
    f8 = np.float64
    out = np.zeros((len(toks), E), f8)
    wq8, wo8 = wq.astype(f8), wo.astype(f8)
    gw18, gw28 = gw1.astype(f8), gw2.astype(f8)
    byb = {}
    for i, t in enumerate(toks):
        byb.setdefault(int(t) // S, []).append((i, int(t) % S))
    for b, items in byb.items():
        idx = np.array([i for i, _ in items])
        sel = np.array([s for _, s in items])
        xb = x[b].astype(f8)
        mu = xb.mean(1, keepdims=True)
        va = xb.var(1, keepdims=True)
        h = (xb - mu) / np.sqrt(va + EPS) * ln1g + ln1b
        h32 = h.astype(np.float32)
        K = (h32 @ wk + bk).astype(f8)
        V = (h32 @ wv + bv).astype(f8)
        q = h[sel] @ wq8 + bq
        ao = np.empty((len(sel), D), f8)
        for hh in range(H):
            g = hh // 2
            sc = q[:, g * HD:(g + 1) * HD] @ K[:, hh * HD:(hh + 1) * HD].T * SCALE
            sc -= sc.max(axis=1, keepdims=True)
            p = np.exp(sc)
            p /= p.sum(axis=1, keepdims=True)
            ao[:, hh * HD:(hh + 1) * HD] = p @ V[:, hh * HD:(hh + 1) * HD]
        x1 = x[b, sel].astype(f8) + ao @ wo8 + bo
        mu2 = x1.mean(1, keepdims=True)
        va2 = x1.var(1, keepdims=True)
        h2 = (x1 - mu2) / np.sqrt(va2 + EPS) * ln2g + ln2b
        out[idx] = np.maximum(h2 @ gw18 + gb1, 0.0) @ gw28 + gb2
    return out


DEBUG_STATS = {}


def _attn_in_maps(x, wq, bq, wk, bk, wv, bv, wo, bo, ln1g, ln1b, ln2g, ln2b,
                  gw1, gb1, gw2, gb2):
    # head-pair permutations
    perm64 = np.concatenate([np.arange(h * HD, (h + 1) * HD)
                             for pr in range(8) for h in (LO[pr], HI[pr])])
    wk_pm, wv_pm, wo_pm = wk[:, perm64], wv[:, perm64], wo[perm64, :]
    bk_pm, bv_pm = bk[perm64], bv[perm64]

    pc = lambda v: v.reshape(-1, P).T            # [c*128] -> [128, c]
    pcs = np.concatenate([pc(ln2g), pc(ln2b)], axis=1)
    # LN folding: W^T h = (W*g)^T x * a + (-mu) * (W^T g) + (W^T b + bias);
    # the last (constant) term must be zero for this kernel build.
    wq_f = wq * ln1g[:, None]
    wk_f = wk_pm * ln1g[:, None]
    wv_f = wv_pm * ln1g[:, None]
    gw1_f = gw1 * ln2g[:, None]
    qg1 = ln1g @ wq
    kg1 = ln1g @ wk_pm
    vg1 = ln1g @ wv_pm
    Gg1 = ln2g @ gw1
    qbT = ln1b @ wq + bq
    kbT = ln1b @ wk_pm + bk_pm
    vbT = ln1b @ wv_pm + bv_pm
    GbT = ln2b @ gw1 + gb1
    for v in (qbT, kbT, vbT, GbT):
        assert np.abs(v).max() < 1e-12, "nonzero fused bias not supported"
    r1t = (np.concatenate([qg1, kg1, vg1])[None, :] * WS).astype(BF16_NP)

    wv_prep = (wv_f * WS).astype(F8_NP).reshape(NJ, 2, P, 2, 512)
    wv_prep = np.ascontiguousarray(wv_prep.transpose(2, 3, 0, 1, 4))
    shared = dict(
        wq_p=_pair_w(wq_f), wk_p=_pair_w(wk_f), wv_p=wv_prep, wo_p=_pair_w(wo_pm),
        gw1_p=np.ascontiguousarray(
            gw1_f.reshape(DC, P, 4, P).transpose(1, 2, 0, 3), np.float32),
        gw2_p=np.ascontiguousarray(
            gw2.reshape(4, P, E).transpose(1, 0, 2), np.float32),
        r1t=np.ascontiguousarray(r1t),
        gg1=np.ascontiguousarray(Gg1[None, :], np.float32),
        pcs=np.ascontiguousarray(pcs, np.float32),
        gb2=np.ascontiguousarray(gb2[:, None]))
    in_maps = []
    x8 = x.astype(F8_NP)                        # fp8 stream of x
    for c in range(NCORES):
        b, half = c // 2, c % 2
        xbT8 = x8[b].T
        xbT = x[b].T
        if half == 1:       # rotate so own tokens come first
            xbT8 = np.concatenate([xbT8[:, SQ:], xbT8[:, :SQ]], axis=1)
            xbT = np.concatenate([xbT[:, SQ:], xbT[:, :SQ]], axis=1)
        xp = np.ascontiguousarray(
            xbT8.reshape(NJ, 2, P, S).transpose(2, 0, 1, 3))
        xqh = (xbT[:, :SQ] + bo[:, None]).astype(BF16_NP)
        xqh = np.ascontiguousarray(xqh.reshape(DC, P, SQ).transpose(1, 0, 2))
        in_maps.append(dict(shared, xp=xp, xq=xqh))
    return in_maps


def kernel(**inputs):
    x = np.ascontiguousarray(np.asarray(inputs["x"], np.float32))
    get = lambda k: np.ascontiguousarray(np.asarray(inputs[k], np.float32))
    wq, wk, wv, wo = get("wq"), get("wk"), get("wv"), get("wo")
    bq, bk, bv, bo = get("bq"), get("bk"), get("bv"), get("bo")
    ln1g, ln1b, ln2g, ln2b = get("ln1_g"), get("ln1_b"), get("ln2_g"), get("ln2_b")
    gw1, gb1, gw2, gb2 = get("gw1"), get("gb1"), get("gw2"), get("gb2")
    ew1, eb1, eb2, ew2 = get("ew1"), get("eb1"), get("eb2"), get("ew2")

    if "attn" not in _CACHE:
        _CACHE["attn"] = build_attn()
    nc1 = _CACHE["attn"]
    in_maps = _attn_in_maps(x, wq, bq, wk, bk, wv, bv, wo, bo,
                            ln1g, ln1b, ln2g, ln2b, gw1, gb1, gw2, gb2)
    r1 = run_bass_kernel_spmd(nc1, in_maps, core_ids=list(range(NCORES)))

    x1 = np.empty((T, D), np.float32)
    h2b = np.empty((T, D), F8_NP)
    glog = np.empty((T, E), np.float32)
    for c in range(NCORES):
        b, half = c // 2, c % 2
        sl = slice(b * S + half * SQ, b * S + (half + 1) * SQ)
        x1[sl] = r1.results[c]["x1T"].transpose(2, 1, 0).reshape(SQ, D)
        h2b[sl] = r1.results[c]["h2T"].transpose(2, 1, 0).reshape(SQ, D)
        glog[sl] = r1.results[c]["glogT"].T

    # ---- routing: softmax -> top-k -> renorm, with exact rescue ------------
    gate_w = _softmax_np(glog)
    srt = np.sort(gate_w, axis=1)
    sus = np.where(srt[:, -2] - srt[:, -3] < SUS_MARGIN)[0]
    DEBUG_STATS["sus"] = len(sus)
    if len(sus):
        glog[sus] = _exact_gate_rows(
            x, wq, bq, wk, bk, wv, bv, wo, bo, ln1g, ln1b, ln2g, ln2b,
            gw1, gb1, gw2, gb2, sus).astype(np.float32)
        gate_w[sus] = _softmax_np(glog[sus])
    idx = np.argsort(-gate_w, axis=1, kind="stable")[:, :TOPK]
    top_w = np.take_along_axis(gate_w, idx, axis=1)
    ren = _softmax_np(top_w)

    tok_lists, wt_lists = [], []
    for e in range(E):
        sel0 = np.where(idx[:, 0] == e)[0]
        sel1 = np.where(idx[:, 1] == e)[0]
        tok_lists.append(np.concatenate([sel0, sel1]))
        wt_lists.append(np.concatenate([ren[sel0, 0], ren[sel1, 1]]).astype(np.float32))

    caps, assignment = _pack_slots(tok_lists, wt_lists)
    DEBUG_STATS["caps"] = caps
    if ("ffn", caps) not in _CACHE:
        _CACHE[("ffn", caps)] = build_ffn(caps)
    nc2 = _CACHE[("ffn", caps)]

    w1_blocks = {e: _pair_w(ew1[e]) for e in range(E)}
    w2_blocks = {e: np.ascontiguousarray(
        ew2[e].astype(BF16_NP).reshape(EC, P, DC, P).transpose(1, 2, 0, 3))
        for e in range(E)}
    in_maps2 = []
    for c in range(NCORES):
        m = {}
        for si, (e, toks, wts) in enumerate(assignment[c]):
            cap = caps[si]
            xgT = np.zeros((P, NJ, 2, cap), F8_NP)
            if len(toks):
                sel = h2b[toks]                       # [n, D] fp8
                xgT[:, :, :, :len(toks)] = (
                    sel.reshape(-1, NJ, 2, P).transpose(3, 1, 2, 0))
            wt_arr = np.zeros((1, cap), np.float32)
            wt_arr[0, :len(toks)] = wts
            m[f"xg{si}"] = np.ascontiguousarray(xgT)
            m[f"w1_{si}"] = w1_blocks[e]
            m[f"w2_{si}"] = w2_blocks[e]
            m[f"eb1_{si}"] = np.ascontiguousarray(eb1[e].reshape(EC, P).T)
            m[f"eb2_{si}"] = np.ascontiguousarray(eb2[e].reshape(DC, P).T)
            m[f"wt{si}"] = wt_arr
        in_maps2.append(m)
    r2 = run_bass_kernel_spmd(nc2, in_maps2, core_ids=list(range(NCORES)))

    moe = np.zeros((T, D), np.float32)
    for c in range(NCORES):
        for si, (e, toks, wts) in enumerate(assignment[c]):
            if len(toks):
                y = r2.results[c][f"y{si}"]           # [128, DC, cap] bf16
                yt = y[:, :, :len(toks)].transpose(2, 1, 0).reshape(len(toks), D)
                moe[toks] += yt.astype(np.float32)

    return (x1 + moe).reshape(B, S, D).astype(np.float32)


# revision 21
# speedup vs baseline: 1.1611x; 1.1611x over previous
"""Trainium2 Bass kernel for nn_CrossModalAttentionBlock (GQA attention + top-2 MoE).

Two SPMD launches over 8 cores:

  L1 "attn" (token-parallel): core c = (batch b=c//2, half=c%2) owns 512 query
    tokens; the host rotates each core's batch sequence so its own half comes
    first. LN1 is folded into the consumers (h = x*a + c per token), so the
    heavy projections run directly on raw x with a rank-1 (-mu * W^T g) matmul
    accumulated into the same psum and a single a-scale on the way out of
    PSUM. Q/K/V/O projections and attn@V run as fp8e4 DoubleRow matmuls (two
    contraction rows per cycle; weights pre-scaled x64 on the host so w~0.02
    stays in the fp8 normal range, the 1/64 folded into the psum post-scale;
    exp writes its fp8 probs directly into the DoubleRow pair slices).
    Scores stay bf16 (two heads packed per PE pass); the gate MLP stays fp32r
    so routing margins are tight.
  Host: top-2 routing mirroring the reference; tokens whose 2nd/3rd gate
    margin is inside the device error envelope are recomputed exactly.
  L2 "moe" (expert-parallel): hidden layer gelu(X@w1) in fp8 DoubleRow,
    out layer @w2 in bf16, scaled by the renormalized gate weight, over
    tokens routed per slot (padded to uniform per-core capacities).
  Host: scatter-add + final residual.

All tensors ship in exactly the layout the engines consume: weights as
[part, ..., pair, 2, cols] DoubleRow stationary blocks, activations as
[part, pair, 2, cols] pair tiles, one large contiguous DMA per tensor,
ordered so the first consumer's bytes land first."""

import numpy as np

import concourse.bass as bass
import concourse.mybir as mybir
import concourse.tile as tile
from concourse import bacc
from concourse.bass_utils import run_bass_kernel_spmd

AF = mybir.ActivationFunctionType
ALU = mybir.AluOpType
FP32 = mybir.dt.float32
FP32R = mybir.dt.float32r
BF16 = mybir.dt.bfloat16
F8 = mybir.dt.float8e4
DR = mybir.MatmulPerfMode.DoubleRow
BF16_NP = mybir.dt.np(BF16)
F8_NP = mybir.dt.np(F8)

B, S, D = 4, 1024, 1024
H, G = 16, 8
HD = D // H              # 64
E, TOPK, ED = 8, 2, 2 * D
GH = D // 2              # 512
EPS = 1e-5
P = 128
NCORES = 8
SQ = S // 2              # 512 query tokens per core
T = B * S
DC = D // P              # 8 feature chunks
NJ = DC // 2             # 4 DoubleRow k-chunk pairs over D
EC = ED // P             # 16 hidden chunks
SCALE = HD ** -0.5
WS = 64.0                # fp8 weight scale
US = 64.0                # fp8 attention-output scale

LO = [0, 1, 4, 5, 8, 9, 12, 13]
HI = [2, 3, 6, 7, 10, 11, 14, 15]
SLOT_HEAD = [h for p in range(8) for h in (LO[p], HI[p])]

# Routing margin below which the host recomputes gate logits exactly.
SUS_MARGIN = 2.5e-3


# ------------------------------------------------------------- host helpers --

def _pair_w(w, scale=WS):
    """[K, M] weight -> [128, M/128, K/256, 2, 128] fp8 DoubleRow blocks."""
    K, M = w.shape
    a = (np.asarray(w, np.float32) * scale).astype(F8_NP)
    a = a.reshape(K // 256, 2, P, M // P, P).transpose(2, 3, 0, 1, 4)
    return np.ascontiguousarray(a)


def _softmax_np(x, axis=-1):
    m = x.max(axis=axis, keepdims=True)
    e = np.exp(x - m)
    return e / e.sum(axis=axis, keepdims=True)


# ------------------------------------------------------------------ L1 attn --

def build_attn():
    nc = bacc.Bacc("TRN2", target_bir_lowering=False, debug=False, num_devices=NCORES)

    xp_d = nc.dram_tensor("xp", [P, NJ, 2, S], F8, kind="ExternalInput").ap()
    xq_d = nc.dram_tensor("xq", [P, DC, SQ], BF16, kind="ExternalInput").ap()
    wq_d = nc.dram_tensor("wq_p", [P, 4, NJ, 2, P], F8, kind="ExternalInput").ap()
    wk_d = nc.dram_tensor("wk_p", [P, DC, NJ, 2, P], F8, kind="ExternalInput").ap()
    wv_d = nc.dram_tensor("wv_p", [P, 2, NJ, 2, 512], F8, kind="ExternalInput").ap()
    wo_d = nc.dram_tensor("wo_p", [P, DC, NJ, 2, P], F8, kind="ExternalInput").ap()
    gw1_d = nc.dram_tensor("gw1_p", [P, 4, DC, P], FP32R, kind="ExternalInput").ap()
    gw2_d = nc.dram_tensor("gw2_p", [P, 4, E], FP32R, kind="ExternalInput").ap()
    # rank-1 row tables bf16 (x WS): qg1[512], kg1[1024], vg1[1024]
    r1_d = nc.dram_tensor("r1t", [1, 512 + D + D], BF16, kind="ExternalInput").ap()
    gg1_d = nc.dram_tensor("gg1", [1, 512], FP32R, kind="ExternalInput").ap()
    pcs_d = nc.dram_tensor("pcs", [P, 2 * DC], FP32, kind="ExternalInput").ap()
    gb2_d = nc.dram_tensor("gb2", [E, 1], FP32, kind="ExternalInput").ap()

    x1T_d = nc.dram_tensor("x1T", [P, DC, SQ], FP32, kind="ExternalOutput").ap()
    h2T_d = nc.dram_tensor("h2T", [P, DC, SQ], F8, kind="ExternalOutput").ap()
    glogT_d = nc.dram_tensor("glogT", [E, SQ], FP32, kind="ExternalOutput").ap()

    with tile.TileContext(nc) as tc:
        import contextlib
        ctx = contextlib.ExitStack()
        with ctx:
            const = ctx.enter_context(tc.tile_pool(name="const", bufs=1))
            rows = ctx.enter_context(tc.tile_pool(name="rows", bufs=2))
            bcast = ctx.enter_context(tc.tile_pool(name="bcast", bufs=2))
            tmp_f = ctx.enter_context(tc.tile_pool(name="tmpf", bufs=2))
            qt_pool = ctx.enter_context(tc.tile_pool(name="qt", bufs=4))
            ut_pool = ctx.enter_context(tc.tile_pool(name="ut", bufs=NJ))
            xin = ctx.enter_context(tc.tile_pool(name="xin", bufs=1))
            wts = ctx.enter_context(tc.tile_pool(name="wts", bufs=1))
            ps_main = ctx.enter_context(tc.tile_pool(name="psm", bufs=2, space="PSUM"))
            ps_sc = ctx.enter_context(tc.tile_pool(name="pssc", bufs=4, space="PSUM"))
            ps_att = ctx.enter_context(tc.tile_pool(name="psat", bufs=2, space="PSUM"))

            # ---- inputs: earliest consumer's bytes first --------------------
            xp = xin.tile([P, NJ, 2, S], F8, tag="xp", name="xp")
            nc.sync.dma_start(xp[:, :, :, 0:512], xp_d[:, :, :, 0:512])
            nc.sync.dma_start(xp[:, :, :, 512:1024], xp_d[:, :, :, 512:1024])
            wk8 = wts.tile([P, DC, NJ, 2, P], F8, tag="wk8", name="wk8")
            nc.scalar.dma_start(wk8[:], wk_d[:])
            wq8 = wts.tile([P, 4, NJ, 2, P], F8, tag="wq8", name="wq8")
            nc.scalar.dma_start(wq8[:], wq_d[:])
            wv8 = wts.tile([P, 2, NJ, 2, 512], F8, tag="wv8", name="wv8")
            wo8 = wts.tile([P, DC, NJ, 2, P], F8, tag="wo8", name="wo8")
            gwb = wts.tile([P, 4, DC, P], FP32R, tag="gw1", name="gw1")
            gw2b = wts.tile([P, 4, E], FP32R, tag="gw2", name="gw2")
            xq = xin.tile([P, DC, SQ], BF16, tag="xq", name="xq")

            # ---- constants -------------------------------------------------
            ones_f = const.tile([P, 1], FP32)
            nc.vector.memset(ones_f[:], 1.0)
            ones_r = const.tile([P, 1], FP32R)
            nc.scalar.copy(ones_r[:], ones_f[:])
            ones_row_f = const.tile([1, P], FP32)
            nc.vector.memset(ones_row_f[:], 1.0)
            ones_row = const.tile([1, P], FP32R)
            nc.scalar.copy(ones_row[:], ones_row_f[:])
            ones_sq_f = const.tile([P, 64], FP32)
            nc.vector.memset(ones_sq_f[:], 1.0)
            ones_sq = const.tile([P, 64], FP32R)
            nc.scalar.copy(ones_sq[:], ones_sq_f[:])
            ones16 = const.tile([1, 1], BF16)
            nc.vector.memset(ones16[:], 1.0)
            ones8_t = const.tile([P, 2, 16], F8)
            nc.vector.memset(ones8_t[:], 1.0)
            ones8 = ones8_t[:, :, 0:1]      # pair-axis step 16 (ISA: step%16==0)
            c4096 = const.tile([P, 1], FP32)
            nc.vector.memset(c4096[:], 1.0 / (WS * US))
            r1t = const.tile([1, 512 + D + D], BF16, tag="r1t", name="r1t")
            nc.sync.dma_start(r1t[:], r1_d[:])
            qg1 = r1t[:, 0:512]
            kg1 = r1t[:, 512:512 + D]
            vg1 = r1t[:, 512 + D:512 + 2 * D]
            Gg1 = const.tile([1, 512], FP32R, tag="gg1", name="gg1")
            nc.sync.dma_start(Gg1[:], gg1_d[:])
            Gg1 = Gg1[:, :]
            pcs = const.tile([P, 2 * DC], FP32, tag="pcs", name="pcs")
            nc.sync.dma_start(pcs[:], pcs_d[:])
            g2_pc = pcs[:, 0:DC]
            b2_pc = pcs[:, DC:2 * DC]
            gb2_pc = const.tile([E, 1], FP32)
            nc.sync.dma_start(gb2_pc[:], gb2_d[:])
            eps_b = const.tile([1, 1], FP32)
            nc.vector.memset(eps_b[:], float(EPS))

            # PE warm-up while the xp DMA lands
            warm = const.tile([P, P], BF16)
            nc.vector.memset(warm[:], 0.0)
            psw = ps_sc.tile([P, P], FP32, tag="ps_s", name="psw")
            for i in range(30):
                nc.tensor.matmul(psw[:], warm[:], warm[:], start=True, stop=True)

            # ---- LN1 stats (window n of 512 tokens) -------------------------
            att_ctx = contextlib.ExitStack()
            sq_pool = att_ctx.enter_context(tc.tile_pool(name="sqp", bufs=1))
            sq8 = sq_pool.tile([P, NJ, 2, S], F8, tag="sq8", name="sq8")

            stats = {}

            def bcast_rows(psum_pool, tag, row, w, name):
                """[1, w] row -> [128, w] sbuf via a K=1 PE matmul + copy."""
                psb = psum_pool.tile([P, w], FP32, tag=tag, name=f"psb_{name}")
                nc.tensor.matmul(psb[:], ones_row[:], row[:],
                                 start=True, stop=True)
                out = bcast.tile([P, w], FP32, tag="a_b", name=f"bc_{name}")
                nc.vector.tensor_copy(out[:], psb[:])
                return out

            def stats_win1(n):
                w = 512
                sl = slice(n * 512, (n + 1) * 512)
                for j in range(NJ):
                    for i in range(2):
                        nc.vector.tensor_tensor(sq8[:, j, i, sl], xp[:, j, i, sl],
                                                xp[:, j, i, sl], ALU.mult)
                psx = ps_main.tile([1, 512], FP32, tag="ps", name=f"psx{n}")
                psq = ps_main.tile([1, 512], FP32, tag="ps", name=f"psq{n}")
                for j in range(NJ):
                    nc.tensor.matmul(psx[:], ones8, xp[:, j, :, sl],
                                     start=(j == 0), stop=(j == NJ - 1), perf_mode=DR)
                for j in range(NJ):
                    nc.tensor.matmul(psq[:], ones8, sq8[:, j, :, sl],
                                     start=(j == 0), stop=(j == NJ - 1), perf_mode=DR)
                mu_row = rows.tile([1, w], FP32, tag="mu", name=f"mu{n}")
                var_row = rows.tile([1, w], FP32, tag="var", name=f"var{n}")
                t_row = rows.tile([1, w], FP32, tag="t", name=f"t{n}")
                nc.vector.tensor_scalar_mul(mu_row[:], psx[:1, :], 1.0 / D)
                nc.vector.tensor_tensor(t_row[:], mu_row[:], mu_row[:], ALU.mult)
                nc.vector.scalar_tensor_tensor(var_row[:], psq[:1, :], 1.0 / D,
                                               t_row[:], ALU.mult, ALU.subtract)
                sd_row = rows.tile([1, w], FP32, tag="t", name=f"sd{n}")
                nc.scalar.activation(sd_row[:], var_row[:], AF.Sqrt, bias=eps_b[:])
                a_row = rows.tile([1, w], FP32, tag="var", name=f"a{n}")
                nc.vector.reciprocal_approx_fast(out=a_row[:], in_=sd_row[:])
                aS_row = rows.tile([1, w], FP32R, tag="as", name=f"as{n}")
                nc.vector.tensor_scalar_mul(aS_row[:], a_row[:], 1.0 / WS)
                nmu_row = rows.tile([1, w], BF16, tag="mu2", name=f"nmu{n}")
                nc.vector.tensor_scalar_mul(nmu_row[:], mu_row[:], -1.0)
                aS16_row = rows.tile([1, w], BF16, tag="as16", name=f"as16_{n}")
                nc.vector.tensor_scalar_mul(aS16_row[:], a_row[:], 1.0 / WS)
                a_bS = bcast_rows(ps_main, "ps", aS_row, w, f"a{n}")
                stats[n] = (a_bS, nmu_row, aS16_row)

            att_ctx2 = contextlib.ExitStack()
            kt_pool = att_ctx2.enter_context(tc.tile_pool(name="kt", bufs=DC))
            va_pool = att_ctx2.enter_context(tc.tile_pool(name="va", bufs=NJ))
            ex_pool = att_ctx2.enter_context(tc.tile_pool(name="ex", bufs=14))
            nrm = att_ctx2.enter_context(tc.tile_pool(name="nrm", bufs=2))

            QT = [qt_pool.tile([P, SQ], BF16, tag="QT", name=f"QT{i}") for i in range(4)]
            KTH = [kt_pool.tile([P, S], BF16, tag="KTH", name=f"KTH{i}") for i in range(DC)]
            # V pairs: [key-part, kc-pair-slice, head-slot, 64 v | 1 ones]
            V_p = [va_pool.tile([P, 2, 16, 65], F8, tag="V_p", name=f"V_p{j}")
                   for j in range(NJ)]
            UTp = [ut_pool.tile([P, 2, SQ], F8, tag="UTp", name=f"UTp{i}")
                   for i in range(NJ)]
            for j in range(NJ):
                nc.vector.memset(V_p[j][:, :, :, 64:65], 1.0)
            acol = const.tile([P, DC], FP32, tag="acol", name="acol")

            def q_proj(ms):
                a_bS, nmu_row = stats[0][0], stats[0][1]
                for m in ms:
                    psq = ps_main.tile([P, 512], FP32, tag="ps", name=f"psQ{m}")
                    for j in range(NJ):
                        nc.tensor.matmul(psq[:], wq8[:, m, j], xp[:, j, :, 0:SQ],
                                         start=(j == 0), stop=False, perf_mode=DR)
                    nc.tensor.matmul(psq[:], qg1[:, m * P:(m + 1) * P], nmu_row[:],
                                     start=False, stop=True, skip_group_check=True)
                    nc.vector.tensor_tensor(QT[m][:], psq[:], a_bS[:], ALU.mult)

            def k_proj(p, n):
                a_bS, nmu_row = stats[n][0], stats[n][1]
                sl = slice(n * 512, (n + 1) * 512)
                psk = ps_main.tile([P, 512], FP32, tag="ps", name=f"psK{p}_{n}")
                for j in range(NJ):
                    nc.tensor.matmul(psk[:], wk8[:, p, j], xp[:, j, :, sl],
                                     start=(j == 0), stop=False, perf_mode=DR)
                nc.tensor.matmul(psk[:], kg1[:, p * P:(p + 1) * P], nmu_row[:],
                                 start=False, stop=True, skip_group_check=True)
                nc.vector.tensor_tensor(KTH[p][:, sl], psk[:], a_bS[:], ALU.mult)

            def v_proj(n, scs):
                for sc in scs:
                    w = sc // 4     # token window of this block
                    nmu_row = stats[w][1]
                    psv = ps_main.tile([P, 512], FP32, tag="ps", name=f"psV{n}_{sc}")
                    for j in range(NJ):
                        nc.tensor.matmul(psv[:], xp[:, j, :, sc * P:(sc + 1) * P],
                                         wv8[:, n, j], start=(j == 0), stop=False,
                                         perf_mode=DR)
                    nc.tensor.matmul(psv[:],
                                     nmu_row[:, (sc % 4) * P:(sc % 4 + 1) * P],
                                     vg1[:, n * 512:(n + 1) * 512],
                                     start=False, stop=True, skip_group_check=True)
                    nc.vector.tensor_scalar(
                        V_p[sc // 2][:, sc % 2, n * 8:(n + 1) * 8, 0:64],
                        psv.rearrange("p (h d) -> p h d", d=64),
                        acol[:, sc:sc + 1], None, ALU.mult)

            def sc_half(p, hi, js=range(NJ)):
                off = hi * 64
                slot = 2 * p + hi
                g = SLOT_HEAD[slot] // 2
                mq, qoff = g // 2, (g % 2) * 64
                assert qoff == off
                expS = sc_half.exp.setdefault(slot, {})
                for j in js:
                    expS[j] = ex_pool.tile([P, 2, SQ], F8, tag="expS",
                                           name=f"expS{slot}_{j}")
                    for i in range(2):
                        kc = 2 * j + i
                        pss = ps_sc.tile([P, 512], FP32, tag="ps_s",
                                         name=f"s{slot}_{kc}")
                        nc.tensor.matmul(pss[:],
                                         KTH[p][off:off + 64, kc * P:(kc + 1) * P],
                                         QT[mq][qoff:qoff + 64, :],
                                         start=True, stop=True)
                        nc.scalar.activation(expS[j][:, i, :], pss[:], AF.Exp,
                                             scale=SCALE)
            sc_half.exp = {}

            def av_mms(p, hi):
                slot = 2 * p + hi
                expS = sc_half.exp.pop(slot)
                psa = ps_att.tile([65, 512], FP32, tag="pa", name=f"a{slot}")
                for j in range(NJ):
                    nc.tensor.matmul(psa[:], V_p[j][:, :, slot, :], expS[j][:],
                                     start=(j == 0), stop=(j == NJ - 1),
                                     perf_mode=DR)
                av_mms.psa[slot] = psa
            av_mms.psa = {}

            def av_norm(p):
                psas = [av_mms.psa.pop(2 * p), av_mms.psa.pop(2 * p + 1)]
                j, i = p // 2, p % 2
                den_sb = nrm.tile([65, 1024], FP32, tag="den", name=f"ds{p}")
                den0 = nrm.tile([1, 1024], FP32, tag="den0", name=f"d{p}")
                for hi in range(2):
                    nc.vector.tensor_copy(den_sb[64:65, hi * 512:(hi + 1) * 512],
                                          psas[hi][64:65, :])
                    nc.sync.dma_start(den0[:, hi * 512:(hi + 1) * 512],
                                      den_sb[64:65, hi * 512:(hi + 1) * 512])
                rec0 = nrm.tile([1, 1024], FP32, tag="rec0", name=f"r{p}")
                nc.vector.reciprocal_approx_fast(out=rec0[:], in_=den0[:])
                recb = nrm.tile([64, 1024], FP32, tag="recb", name=f"rb{p}")
                nc.gpsimd.partition_broadcast(recb[:], rec0[:])
                nc.vector.scalar_tensor_tensor(UTp[j][0:64, i, :], psas[0][0:64, :],
                                               US, recb[:, 0:512], ALU.mult, ALU.mult)
                nb = nrm.tile([64, 512], F8, tag="nb", name=f"nb{p}")
                nc.vector.scalar_tensor_tensor(nb[:], psas[1][0:64, :],
                                               US, recb[:, 512:1024], ALU.mult, ALU.mult)
                nc.sync.dma_start(UTp[j][64:128, i, :], nb[:])

            # ---- schedule --------------------------------------------------
            def acols(n):
                for sc in range(n * 4, n * 4 + 4):
                    aS16_row = stats[n][2]
                    ptp = ps_main.tile([P, 1], FP32, tag="ps", name=f"tp{sc}")
                    nc.tensor.matmul(ptp[:], aS16_row[:, (sc % 4) * P:(sc % 4 + 1) * P],
                                     ones16[0:1, :], start=True, stop=True)
                    nc.vector.tensor_copy(acol[:, sc:sc + 1], ptp[:])

            stats_win1(0)
            stats_win1(1)
            nc.scalar.dma_start(wv8[:], wv_d[:])
            acols(0)
            acols(1)
            q_proj([0])
            k_proj(0, 0)
            k_proj(0, 1)
            sc_half(0, 0, [0, 1])
            q_proj([1])
            sc_half(0, 0, [2, 3])
            q_proj([2])
            sc_half(0, 1, [0, 1])
            q_proj([3])
            sc_half(0, 1, [2, 3])
            k_proj(1, 0)
            v_proj(0, range(2))
            k_proj(1, 1)
            v_proj(0, range(2, 4))
            k_proj(2, 0)
            v_proj(0, range(4, 6))
            k_proj(2, 1)
            v_proj(0, range(6, 8))
            # fine-grained interleave: AV of pair p, scores of pair p+1 and
            # K/V fillers for later pairs share the PE queue so a dependent
            # cluster cannot stall it for long.
            for p in range(8):
                nxt = p + 1 < 8
                if nxt:
                    sc_half(p + 1, 0, [0])
                av_mms(p, 0)
                if nxt:
                    sc_half(p + 1, 0, [1])
                av_mms(p, 1)
                if nxt:
                    sc_half(p + 1, 0, [2])
                av_norm(p)
                if p == 0:
                    # late-needed weights ride the gpsimd (SWDGE) ring once
                    # the early loads are done
                    nc.gpsimd.dma_start(wo8[:], wo_d[:])
                    nc.gpsimd.dma_start(gwb[:], gw1_d[:])
                    nc.gpsimd.dma_start(gw2b[:], gw2_d[:])
                    nc.gpsimd.dma_start(xq[:], xq_d[:])
                    v_proj(1, range(2))
                if nxt:
                    sc_half(p + 1, 0, [3])
                if p == 0:
                    v_proj(1, range(2, 4))
                elif p == 1:
                    v_proj(1, range(4, 6))
                if nxt:
                    sc_half(p + 1, 1, [0])
                if p == 1:
                    v_proj(1, range(6, 8))
                if p + 3 <= 7:
                    k_proj(p + 3, 0)
                if nxt:
                    sc_half(p + 1, 1, [1])
                if p + 3 <= 7:
                    k_proj(p + 3, 1)
                if nxt:
                    sc_half(p + 1, 1, [2])
                    sc_half(p + 1, 1, [3])
            att_ctx2.close()     # free KTH/V_p/expS space for the tail
            att_ctx.close()      # free sq8

            # late pools, in space vacated by the attention working set
            x1_pool = ctx.enter_context(tc.tile_pool(name="x1", bufs=1))
            h2_pool = ctx.enter_context(tc.tile_pool(name="h2", bufs=2))
            gh_pool = ctx.enter_context(tc.tile_pool(name="gh", bufs=4))
            sq2_pool = ctx.enter_context(tc.tile_pool(name="sq2", bufs=2))

            # ---- tail: out-projection + residual, LN2, folded gate ---------
            x1T = x1_pool.tile([P, DC, SQ], FP32R, tag="x1T", name="x1T")
            h2T = h2_pool.tile([P, DC, SQ], F8, tag="h2T", name="h2T")
            GhT = [gh_pool.tile([P, SQ], FP32R, tag="GhT", name=f"GhT{i}")
                   for i in range(4)]
            glog_sb = rows.tile([E, SQ], FP32, tag="glog", name="glog")
            st2 = {}

            def out_proj(w):
                sl = slice(w * 256, (w + 1) * 256)
                for m in range(DC):
                    pso = ps_main.tile([P, 256], FP32, tag="ps", name=f"psO{w}_{m}")
                    for j in range(NJ):
                        nc.tensor.matmul(pso[:], wo8[:, m, j], UTp[j][:, :, sl],
                                         start=(j == 0), stop=(j == NJ - 1),
                                         perf_mode=DR)
                    nc.vector.scalar_tensor_tensor(x1T[:, m, sl], pso[:], c4096[:],
                                                   xq[:, m, sl], ALU.mult, ALU.add)
                nc.sync.dma_start(x1T_d[:, :, sl], x1T.bitcast(FP32)[:, :, sl])

            def stats_win2(w):
                sl = slice(w * 256, (w + 1) * 256)
                ww = 256
                psx = ps_att.tile([1, 256], FP32, tag="pa", name=f"psx2_{w}")
                psq = ps_att.tile([1, 256], FP32, tag="pa", name=f"psq2_{w}")
                for kd in range(DC):
                    sq = sq2_pool.tile([P, 256], FP32R, tag="sqt", name=f"sq2_{w}_{kd}")
                    nc.vector.tensor_tensor(sq[:], x1T[:, kd, sl], x1T[:, kd, sl],
                                            ALU.mult)
                    nc.tensor.matmul(psx[:1, :], ones_r[:], x1T[:, kd, sl],
                                     start=(kd == 0), stop=(kd == DC - 1))
                    nc.tensor.matmul(psq[:1, :], ones_r[:], sq[:],
                                     start=(kd == 0), stop=(kd == DC - 1))
                mu_row = rows.tile([1, ww], FP32, tag="mu", name=f"mu2{w}")
                var_row = rows.tile([1, ww], FP32, tag="var", name=f"var2{w}")
                t_row = rows.tile([1, ww], FP32, tag="t", name=f"t2{w}")
                nc.vector.tensor_scalar_mul(mu_row[:], psx[:1, :], 1.0 / D)
                nc.vector.tensor_tensor(t_row[:], mu_row[:], mu_row[:], ALU.mult)
                nc.vector.scalar_tensor_tensor(var_row[:], psq[:1, :], 1.0 / D,
                                               t_row[:], ALU.mult, ALU.subtract)
                sd_row = rows.tile([1, ww], FP32, tag="t", name=f"sd2{w}")
                nc.scalar.activation(sd_row[:], var_row[:], AF.Sqrt, bias=eps_b[:])
                a_row = rows.tile([1, ww], FP32, tag="var", name=f"a2{w}")
                nc.vector.reciprocal_approx_fast(out=a_row[:], in_=sd_row[:])
                a_rowr = rows.tile([1, ww], FP32R, tag="ar", name=f"a2r{w}")
                nc.vector.tensor_scalar_mul(a_rowr[:], a_row[:], 1.0)
                nmu_row = rows.tile([1, ww], FP32R, tag="mu2", name=f"nmu2{w}")
                nc.vector.tensor_scalar_mul(nmu_row[:], mu_row[:], -1.0)
                c_row = rows.tile([1, ww], FP32R, tag="c", name=f"c2{w}")
                nc.vector.tensor_tensor(c_row[:], nmu_row.bitcast(FP32)[:],
                                        a_row[:], ALU.mult)
                a_b = bcast_rows(ps_att, "pa", a_rowr, ww, f"a2_{w}")
                c_b = bcast_rows(ps_att, "pa", c_row, ww, f"c2_{w}")
                st2[w] = (a_b, nmu_row, c_b)

            def gate_win(w):
                a2_b, nmu2_row = st2[w][0], st2[w][1]
                sl = slice(w * 256, (w + 1) * 256)
                for m in range(4):
                    psg = ps_sc.tile([P, 256], FP32, tag="ps_s", name=f"psG{w}_{m}")
                    for kd in range(DC):
                        nc.tensor.matmul(psg[:], gwb[:, m, kd, :], x1T[:, kd, sl],
                                         start=(kd == 0), stop=False)
                    nc.tensor.matmul(psg[:], Gg1[:, m * P:(m + 1) * P],
                                     nmu2_row[:], start=False, stop=True)
                    pre = h2_pool.tile([P, 256], FP32, tag="pre", name=f"pre{w}_{m}")
                    nc.vector.tensor_tensor(pre[:], psg[:], a2_b[:], ALU.mult)
                    nc.vector.tensor_scalar_max(GhT[m][:, sl], pre[:], 0.0)
                psl = ps_att.tile([E, 256], FP32, tag="pa", name=f"psl{w}")
                for gt in range(4):
                    nc.tensor.matmul(psl[:], gw2b[:, gt, :], GhT[gt][:, sl],
                                     start=(gt == 0), stop=(gt == 3))
                nc.vector.tensor_scalar(glog_sb[:, sl], psl[:], gb2_pc[:], None,
                                        ALU.add)

            def h2_win(w):
                sl = slice(w * 256, (w + 1) * 256)
                a2_b, c2_b = st2[w][0], st2[w][2]
                for kd in range(DC):
                    t = tmp_f.tile([P, 256], FP32, tag="lnt2", name=f"l2_{w}_{kd}")
                    nc.vector.tensor_tensor(t[:], x1T[:, kd, sl], a2_b[:], ALU.mult)
                    nc.vector.tensor_tensor(t[:], t[:], c2_b[:], ALU.add)
                    nc.vector.tensor_scalar(h2T[:, kd, sl], t[:],
                                            g2_pc[:, kd:kd + 1],
                                            b2_pc[:, kd:kd + 1], ALU.mult, ALU.add)
                nc.sync.dma_start(h2T_d[:, :, sl], h2T[:, :, sl])

            out_proj(0)
            stats_win2(0)
            out_proj(1)
            gate_win(0)
            stats_win2(1)
            gate_win(1)
            nc.sync.dma_start(glogT_d[:], glog_sb[:])
            h2_win(0)
            h2_win(1)

    nc.compile()
    return nc


# ------------------------------------------------------------------- L2 moe --

def _windows(cap):
    ws = [512] * (cap // 512)
    if cap % 512:
        ws.append(cap % 512)
    return ws


def build_ffn(caps):
    """One FFN slot per entry in `caps` (uniform shapes across cores).
    Both layers fp8 DoubleRow; the renormalized gate weight is pre-divided
    by the w2 scale on the host so the psum drain stays a single op."""
    nc = bacc.Bacc("TRN2", target_bir_lowering=False, debug=False, num_devices=NCORES)

    ins, outs = [], []
    for si, cap in enumerate(caps):
        ins.append(dict(
            xg=nc.dram_tensor(f"xg{si}", [P, NJ, 2, cap], F8, kind="ExternalInput").ap(),
            w1=nc.dram_tensor(f"w1_{si}", [P, EC, NJ, 2, P], F8,
                              kind="ExternalInput").ap(),
            w2=nc.dram_tensor(f"w2_{si}", [P, DC, EC // 2, 2, P], F8,
                              kind="ExternalInput").ap(),
            eb1=nc.dram_tensor(f"eb1_{si}", [P, EC], FP32, kind="ExternalInput").ap(),
            eb2=nc.dram_tensor(f"eb2_{si}", [P, DC], FP32, kind="ExternalInput").ap(),
            wt=nc.dram_tensor(f"wt{si}", [1, cap], FP32, kind="ExternalInput").ap(),
        ))
        outs.append(nc.dram_tensor(f"y{si}", [P, DC, cap], BF16,
                                   kind="ExternalOutput").ap())

    with tile.TileContext(nc) as tc:
        import contextlib
        ctx = contextlib.ExitStack()
        with ctx:
            const = ctx.enter_context(tc.tile_pool(name="const", bufs=1))
            xg_pool = ctx.enter_context(tc.tile_pool(name="xg", bufs=1))
            hid_pool = ctx.enter_context(tc.tile_pool(name="hid", bufs=1))
            w_pool = ctx.enter_context(tc.tile_pool(name="wp", bufs=1))
            out_pool = ctx.enter_context(tc.tile_pool(name="out", bufs=1))
            ps = ctx.enter_context(tc.tile_pool(name="ps", bufs=6, space="PSUM"))

            # sync queue: small/early tensors in consumer order; the big w2
            # transfers are issued mid-stream (scalar queue) so they don't
            # starve the layer-1 inputs.
            # Inputs are spread over the three DMA queue rings (sync /
            # scalar HWDGE + gpsimd SWDGE) in consumer order: each ring
            # drains FIFO, so a big late-needed transfer queued ahead of an
            # early one starves the whole launch.
            tls = []
            for si, cap in enumerate(caps):
                io = ins[si]
                xgt = xg_pool.tile([P, NJ, 2, cap], F8, tag=f"xg_{si}",
                                   name=f"xg_{si}")
                w1t = w_pool.tile([P, EC, NJ, 2, P], F8, tag=f"w1_{si}",
                                  name=f"w1_{si}")
                eb1_pc = const.tile([P, EC], FP32, tag="eb1", name=f"eb1_{si}")
                nc.scalar.dma_start(eb1_pc[:], io["eb1"][:])
                eb2_pc = const.tile([P, DC], FP32, tag="eb2", name=f"eb2_{si}")
                nc.scalar.dma_start(eb2_pc[:], io["eb2"][:])
                wt_row = const.tile([1, caps[0]], FP32, tag="wtr", name=f"wtr{si}")
                nc.scalar.dma_start(wt_row[:1, :cap], io["wt"][:])
                wt_b = const.tile([P, caps[0]], FP32, tag="wtb", name=f"wtb{si}")
                nc.gpsimd.partition_broadcast(wt_b[:, :cap], wt_row[:1, :cap])
                w2t = w_pool.tile([P, DC, EC // 2, 2, P], F8, tag=f"w2_{si}",
                                  name=f"w2_{si}")
                tls.append((w1t, xgt, w2t, eb1_pc, eb2_pc, wt_b))
            nc.sync.dma_start(tls[0][1][:], ins[0]["xg"][:])
            nc.sync.dma_start(tls[0][0][:, 0:EC // 2], ins[0]["w1"][:, 0:EC // 2])
            nc.scalar.dma_start(tls[0][0][:, EC // 2:EC], ins[0]["w1"][:, EC // 2:EC])
            nc.sync.dma_start(tls[0][2][:], ins[0]["w2"][:])
            nc.gpsimd.dma_start(tls[1][2][:], ins[1]["w2"][:])

            # PE warm-up while the first DMAs land
            warm = const.tile([P, P], BF16)
            nc.vector.memset(warm[:], 0.0)
            psw = ps.tile([P, P], FP32, tag="ph", name="psw")
            for i in range(60):
                nc.tensor.matmul(psw[:], warm[:], warm[:], start=True, stop=True)

            for si, cap in enumerate(caps):
                w1t, xgt, w2t, eb1_pc, eb2_pc, wt_b = tls[si]
                io = ins[si]
                WSl = _windows(cap)
                OFF = [sum(WSl[:i]) for i in range(len(WSl))]

                hidT = hid_pool.tile([P, EC // 2, 2, cap], F8, tag=f"hidT{si}",
                                     name=f"hidT{si}")
                for wi, w in enumerate(WSl):
                    sl = slice(OFF[wi], OFF[wi] + w)
                    for ec in range(EC):
                        ph = ps.tile([P, w], FP32, tag="ph", name=f"ph{si}_{ec}_{wi}")
                        for j in range(NJ):
                            nc.tensor.matmul(ph[:], w1t[:, ec, j], xgt[:, j, :, sl],
                                             start=(j == 0), stop=(j == NJ - 1),
                                             perf_mode=DR)
                        nc.scalar.activation(hidT[:, ec // 2, ec % 2, sl], ph[:],
                                             AF.Gelu, bias=eb1_pc[:, ec:ec + 1],
                                             scale=1.0 / WS)
                        if wi == 0 and si == 0 and ec == 1:
                            # slot-1 layer-1 inputs load while slot-0 computes
                            nc.scalar.dma_start(tls[1][1][:], ins[1]["xg"][:])
                            nc.scalar.dma_start(tls[1][0][:], ins[1]["w1"][:])

                ostage = out_pool.tile([P, DC, cap], BF16, tag=f"os{si}",
                                       name=f"os{si}")
                for wi, w in enumerate(WSl):
                    sl = slice(OFF[wi], OFF[wi] + w)
                    for m in range(DC):
                        py = ps.tile([P, w], FP32, tag="ph", name=f"py{si}_{m}_{wi}")
                        for jj in range(EC // 2):
                            nc.tensor.matmul(py[:], w2t[:, m, jj], hidT[:, jj, :, sl],
                                             start=(jj == 0), stop=(jj == EC // 2 - 1),
                                             perf_mode=DR)
                        nc.vector.scalar_tensor_tensor(ostage[:, m, sl], py[:],
                                                       eb2_pc[:, m:m + 1],
                                                       wt_b[:, sl], ALU.add, ALU.mult)
                nc.sync.dma_start(outs[si][:], ostage[:])

    nc.compile()
    return nc


def _pack_slots(tok_lists, wt_lists):
    """Cut per-expert token lists into at most 8 slot-1 pieces (<= c1) and 8
    slot-2 pieces (<= c2), minimizing the uniform SPMD capacities c1 + c2."""
    loads = [len(t) for t in tok_lists]
    act = [e for e in range(len(loads)) if loads[e] > 0]

    def feas(c1, c2):
        n1 = {e: 0 for e in act}
        n2 = {e: -(-loads[e] // c2) for e in act}
        for _ in range(64):
            if sum(n1.values()) > NCORES:
                return None
            if sum(n2.values()) <= NCORES:
                return n1, n2
            def gain(e):
                rem = loads[e] - n1[e] * c1
                if rem <= 0:
                    return (-1, 0)
                new = -(-max(0, rem - c1) // c2)
                return (n2[e] - new, rem)
            e = max(act, key=gain)
            if gain(e)[0] <= 0:
                return None
            n1[e] += 1
            n2[e] = -(-max(0, loads[e] - n1[e] * c1) // c2)
        return None

    best = None
    for c1 in range(512, 3392, 32):
        if best is not None and best[0] <= c1 + 256:
            break
        for c2 in range(256, c1 + 32, 32):
            if best is not None and c1 + c2 >= best[0]:
                break
            r = feas(c1, c2)
            if r is not None:
                best = (c1 + c2, c1, c2, r[0], r[1])
    _, c1, c2, n1, n2 = best
    s1, s2 = [], []
    for e in act:
        off = 0
        for _ in range(n1[e]):
            sz = min(c1, loads[e] - off)
            s1.append((e, off, sz))
            off += sz
        rem = loads[e] - off
        if rem > 0:
            psz = -(-rem // n2[e])
            for _ in range(n2[e]):
                sz = min(psz, loads[e] - off)
                if sz > 0:
                    s2.append((e, off, sz))
                    off += sz
    assert len(s1) <= NCORES and len(s2) <= NCORES
    assignment = []
    for core in range(NCORES):
        slots = []
        for group in (s1, s2):
            if core < len(group):
                e, off, sz = group[core]
                slots.append((e, tok_lists[e][off:off + sz], wt_lists[e][off:off + sz]))
            else:
                slots.append((0, np.zeros(0, np.int64), np.zeros(0, np.float32)))
        assignment.append(slots)
    return (c1, c2), assignment


# --------------------------------------------------------------- host logic --

_CACHE = {}


def _exact_gate_rows(x, wq, bq, wk, bk, wv, bv, wo, bo, ln1g, ln1b, ln2g, ln2b,
                     gw1, gb1, gw2, gb2, toks):
    """Exact (float64, vectorized) gate logits for the given flat token ids."""
    f8 = np.float64
    out = np.zeros((len(toks), E), f8)
    wq8, wo8 = wq.astype(f8), wo.astype(f8)
    gw18, gw28 = gw1.astype(f8), gw2.astype(f8)
    byb = {}
    for i, t in enumerate(toks):
        byb.setdefault(int(t) // S, []).append((i, int(t) % S))
    for b, items in byb.items():
        idx = np.array([i for i, _ in items])
        sel = np.array([s for _, s in items])
        xb = x[b].astype(f8)
        mu = xb.mean(1, keepdims=True)
        va = xb.var(1, keepdims=True)
        h = (xb - mu) / np.sqrt(va + EPS) * ln1g + ln1b
        h32 = h.astype(np.float32)
        K = (h32 @ wk + bk).astype(f8)
        V = (h32 @ wv + bv).astype(f8)
        q = h[sel] @ wq8 + bq
        ao = np.empty((len(sel), D), f8)
        for hh in range(H):
            g = hh // 2
            sc = q[:, g * HD:(g + 1) * HD] @ K[:, hh * HD:(hh + 1) * HD].T * SCALE
            sc -= sc.max(axis=1, keepdims=True)
            p = np.exp(sc)
            p /= p.sum(axis=1, keepdims=True)
            ao[:, hh * HD:(hh + 1) * HD] = p @ V[:, hh * HD:(hh + 1) * HD]
        x1 = x[b, sel].astype(f8) + ao @ wo8 + bo
        mu2 = x1.mean(1, keepdims=True)
        va2 = x1.var(1, keepdims=True)
        h2 = (x1 - mu2) / np.sqrt(va2 + EPS) * ln2g + ln2b
        out[idx] = np.maximum(h2 @ gw18 + gb1, 0.0) @ gw28 + gb2
    return out


DEBUG_STATS = {}


def _attn_in_maps(x, wq, bq, wk, bk, wv, bv, wo, bo, ln1g, ln1b, ln2g, ln2b,
                  gw1, gb1, gw2, gb2):
    # head-pair permutations
    perm64 = np.concatenate([np.arange(h * HD, (h + 1) * HD)
                             for pr in range(8) for h in (LO[pr], HI[pr])])
    wk_pm, wv_pm, wo_pm = wk[:, perm64], wv[:, perm64], wo[perm64, :]
    bk_pm, bv_pm = bk[perm64], bv[perm64]

    pc = lambda v: v.reshape(-1, P).T            # [c*128] -> [128, c]
    pcs = np.concatenate([pc(ln2g), pc(ln2b)], axis=1)
    # LN folding: W^T h = (W*g)^T x * a + (-mu) * (W^T g) + (W^T b + bias);
    # the last (constant) term must be zero for this kernel build.
    wq_f = wq * ln1g[:, None]
    wk_f = wk_pm * ln1g[:, None]
    wv_f = wv_pm * ln1g[:, None]
    gw1_f = gw1 * ln2g[:, None]
    qg1 = ln1g @ wq
    kg1 = ln1g @ wk_pm
    vg1 = ln1g @ wv_pm
    Gg1 = ln2g @ gw1
    qbT = ln1b @ wq + bq
    kbT = ln1b @ wk_pm + bk_pm
    vbT = ln1b @ wv_pm + bv_pm
    GbT = ln2b @ gw1 + gb1
    for v in (qbT, kbT, vbT, GbT):
        assert np.abs(v).max() < 1e-12, "nonzero fused bias not supported"
    r1t = (np.concatenate([qg1, kg1, vg1])[None, :] * WS).astype(BF16_NP)

    wv_prep = (wv_f * WS).astype(F8_NP).reshape(NJ, 2, P, 2, 512)
    wv_prep = np.ascontiguousarray(wv_prep.transpose(2, 3, 0, 1, 4))
    shared = dict(
        wq_p=_pair_w(wq_f), wk_p=_pair_w(wk_f), wv_p=wv_prep, wo_p=_pair_w(wo_pm),
        gw1_p=np.ascontiguousarray(
            gw1_f.reshape(DC, P, 4, P).transpose(1, 2, 0, 3), np.float32),
        gw2_p=np.ascontiguousarray(
            gw2.reshape(4, P, E).transpose(1, 0, 2), np.float32),
        r1t=np.ascontiguousarray(r1t),
        gg1=np.ascontiguousarray(Gg1[None, :], np.float32),
        pcs=np.ascontiguousarray(pcs, np.float32),
        gb2=np.ascontiguousarray(gb2[:, None]))
    in_maps = []
    x8 = x.astype(F8_NP)                        # fp8 stream of x
    for c in range(NCORES):
        b, half = c // 2, c % 2
        xbT8 = x8[b].T
        xbT = x[b].T
        if half == 1:       # rotate so own tokens come first
            xbT8 = np.concatenate([xbT8[:, SQ:], xbT8[:, :SQ]], axis=1)
            xbT = np.concatenate([xbT[:, SQ:], xbT[:, :SQ]], axis=1)
        xp = np.ascontiguousarray(
            xbT8.reshape(NJ, 2, P, S).transpose(2, 0, 1, 3))
        xqh = (xbT[:, :SQ] + bo[:, None]).astype(BF16_NP)
        xqh = np.ascontiguousarray(xqh.reshape(DC, P, SQ).transpose(1, 0, 2))
        in_maps.append(dict(shared, xp=xp, xq=xqh))
    return in_maps


def kernel(**inputs):
    x = np.ascontiguousarray(np.asarray(inputs["x"], np.float32))
    get = lambda k: np.ascontiguousarray(np.asarray(inputs[k], np.float32))
    wq, wk, wv, wo = get("wq"), get("wk"), get("wv"), get("wo")
    bq, bk, bv, bo = get("bq"), get("bk"), get("bv"), get("bo")
    ln1g, ln1b, ln2g, ln2b = get("ln1_g"), get("ln1_b"), get("ln2_g"), get("ln2_b")
    gw1, gb1, gw2, gb2 = get("gw1"), get("gb1"), get("gw2"), get("gb2")
    ew1, eb1, eb2, ew2 = get("ew1"), get("eb1"), get("eb2"), get("ew2")

    if "attn" not in _CACHE:
        _CACHE["attn"] = build_attn()
    nc1 = _CACHE["attn"]
    in_maps = _attn_in_maps(x, wq, bq, wk, bk, wv, bv, wo, bo,
                            ln1g, ln1b, ln2g, ln2b, gw1, gb1, gw2, gb2)
    r1 = run_bass_kernel_spmd(nc1, in_maps, core_ids=list(range(NCORES)))

    x1 = np.empty((T, D), np.float32)
    h2b = np.empty((T, D), F8_NP)
    glog = np.empty((T, E), np.float32)
    for c in range(NCORES):
        b, half = c // 2, c % 2
        sl = slice(b * S + half * SQ, b * S + (half + 1) * SQ)
        x1[sl] = r1.results[c]["x1T"].transpose(2, 1, 0).reshape(SQ, D)
        h2b[sl] = r1.results[c]["h2T"].transpose(2, 1, 0).reshape(SQ, D)
        glog[sl] = r1.results[c]["glogT"].T

    # ---- routing: softmax -> top-k -> renorm, with exact rescue ------------
    gate_w = _softmax_np(glog)
    srt = np.sort(gate_w, axis=1)
    sus = np.where(srt[:, -2] - srt[:, -3] < SUS_MARGIN)[0]
    DEBUG_STATS["sus"] = len(sus)
    if len(sus):
        glog[sus] = _exact_gate_rows(
            x, wq, bq, wk, bk, wv, bv, wo, bo, ln1g, ln1b, ln2g, ln2b,
            gw1, gb1, gw2, gb2, sus).astype(np.float32)
        gate_w[sus] = _softmax_np(glog[sus])
    idx = np.argsort(-gate_w, axis=1, kind="stable")[:, :TOPK]
    top_w = np.take_along_axis(gate_w, idx, axis=1)
    ren = _softmax_np(top_w)

    tok_lists, wt_lists = [], []
    for e in range(E):
        sel0 = np.where(idx[:, 0] == e)[0]
        sel1 = np.where(idx[:, 1] == e)[0]
        tok_lists.append(np.concatenate([sel0, sel1]))
        wt_lists.append(np.concatenate([ren[sel0, 0], ren[sel1, 1]]).astype(np.float32))

    caps, assignment = _pack_slots(tok_lists, wt_lists)
    DEBUG_STATS["caps"] = caps
    if ("ffn", caps) not in _CACHE:
        _CACHE[("ffn", caps)] = build_ffn(caps)
    nc2 = _CACHE[("ffn", caps)]

    w1_blocks = {e: _pair_w(ew1[e]) for e in range(E)}
    w2_blocks = {e: _pair_w(ew2[e]) for e in range(E)}
    in_maps2 = []
    for c in range(NCORES):
        m = {}
        for si, (e, toks, wts) in enumerate(assignment[c]):
            cap = caps[si]
            xgT = np.zeros((P, NJ, 2, cap), F8_NP)
            if len(toks):
                sel = h2b[toks]                       # [n, D] fp8
                xgT[:, :, :, :len(toks)] = (
                    sel.reshape(-1, NJ, 2, P).transpose(3, 1, 2, 0))
            wt_arr = np.zeros((1, cap), np.float32)
            wt_arr[0, :len(toks)] = wts
            m[f"xg{si}"] = np.ascontiguousarray(xgT)
            m[f"w1_{si}"] = w1_blocks[e]
            m[f"w2_{si}"] = w2_blocks[e]
            m[f"eb1_{si}"] = np.ascontiguousarray(eb1[e].reshape(EC, P).T)
            m[f"eb2_{si}"] = np.ascontiguousarray(eb2[e].reshape(DC, P).T * WS)
            m[f"wt{si}"] = wt_arr / WS
        in_maps2.append(m)
    r2 = run_bass_kernel_spmd(nc2, in_maps2, core_ids=list(range(NCORES)))

    moe = np.zeros((T, D), np.float32)
    for c in range(NCORES):
        for si, (e, toks, wts) in enumerate(assignment[c]):
            if len(toks):
                y = r2.results[c][f"y{si}"]           # [128, DC, cap] bf16
                yt = y[:, :, :len(toks)].transpose(2, 1, 0).reshape(len(toks), D)
                moe[toks] += yt.astype(np.float32)

    return (x1 + moe).reshape(B, S, D).astype(np.float32)


# revision 22
# speedup vs baseline: 1.2483x; 1.0751x over previous
"""Trainium2 Bass kernel for nn_CrossModalAttentionBlock (GQA attention + top-2 MoE).

Two SPMD launches over 8 cores:

  L1 "attn" (token-parallel): core c = (batch b=c//2, half=c%2) owns 512 query
    tokens; the host rotates each core's batch sequence so its own half comes
    first. LN1 is folded into the consumers (h = x*a + c per token), so the
    heavy projections run directly on raw x with a rank-1 (-mu * W^T g) matmul
    accumulated into the same psum and a single a-scale on the way out of
    PSUM. Q/K/V/O projections and attn@V run as fp8e4 DoubleRow matmuls (two
    contraction rows per cycle; weights pre-scaled x64 on the host so w~0.02
    stays in the fp8 normal range, the 1/64 folded into the psum post-scale;
    exp writes its fp8 probs directly into the DoubleRow pair slices).
    Scores stay bf16 (two heads packed per PE pass); the gate MLP stays fp32r
    so routing margins are tight.
  Host: top-2 routing mirroring the reference; tokens whose 2nd/3rd gate
    margin is inside the device error envelope are recomputed exactly.
  L2 "moe" (expert-parallel): hidden layer gelu(X@w1) in fp8 DoubleRow,
    out layer @w2 in bf16, scaled by the renormalized gate weight, over
    tokens routed per slot (padded to uniform per-core capacities).
  Host: scatter-add + final residual.

All tensors ship in exactly the layout the engines consume: weights as
[part, ..., pair, 2, cols] DoubleRow stationary blocks, activations as
[part, pair, 2, cols] pair tiles, one large contiguous DMA per tensor,
ordered so the first consumer's bytes land first."""

import numpy as np

import concourse.bass as bass
import concourse.mybir as mybir
import concourse.tile as tile
from concourse import bacc
from concourse.bass_utils import run_bass_kernel_spmd

AF = mybir.ActivationFunctionType
ALU = mybir.AluOpType
FP32 = mybir.dt.float32
FP32R = mybir.dt.float32r
BF16 = mybir.dt.bfloat16
F8 = mybir.dt.float8e4
DR = mybir.MatmulPerfMode.DoubleRow
BF16_NP = mybir.dt.np(BF16)
F8_NP = mybir.dt.np(F8)

B, S, D = 4, 1024, 1024
H, G = 16, 8
HD = D // H              # 64
E, TOPK, ED = 8, 2, 2 * D
GH = D // 2              # 512
EPS = 1e-5
P = 128
NCORES = 8
SQ = S // 2              # 512 query tokens per core
T = B * S
DC = D // P              # 8 feature chunks
NJ = DC // 2             # 4 DoubleRow k-chunk pairs over D
EC = ED // P             # 16 hidden chunks
SCALE = HD ** -0.5
WS = 64.0                # fp8 weight scale
US = 64.0                # fp8 attention-output scale

LO = [0, 1, 4, 5, 8, 9, 12, 13]
HI = [2, 3, 6, 7, 10, 11, 14, 15]
SLOT_HEAD = [h for p in range(8) for h in (LO[p], HI[p])]

# Routing margin below which the host recomputes gate logits exactly.
SUS_MARGIN = 2.5e-3


# ------------------------------------------------------------- host helpers --

def _pair_w(w, scale=WS):
    """[K, M] weight -> [128, M/128, K/256, 2, 128] fp8 DoubleRow blocks."""
    K, M = w.shape
    a = (np.asarray(w, np.float32) * scale).astype(F8_NP)
    a = a.reshape(K // 256, 2, P, M // P, P).transpose(2, 3, 0, 1, 4)
    return np.ascontiguousarray(a)


def _softmax_np(x, axis=-1):
    m = x.max(axis=axis, keepdims=True)
    e = np.exp(x - m)
    return e / e.sum(axis=axis, keepdims=True)


# ------------------------------------------------------------------ L1 attn --

def build_attn():
    nc = bacc.Bacc("TRN2", target_bir_lowering=False, debug=False, num_devices=NCORES)

    xp_d = nc.dram_tensor("xp", [P, NJ, 2, S], F8, kind="ExternalInput").ap()
    xq_d = nc.dram_tensor("xq", [P, DC, SQ], BF16, kind="ExternalInput").ap()
    wq_d = nc.dram_tensor("wq_p", [P, 4, NJ, 2, P], F8, kind="ExternalInput").ap()
    wk_d = nc.dram_tensor("wk_p", [P, DC, NJ, 2, P], F8, kind="ExternalInput").ap()
    wv_d = nc.dram_tensor("wv_p", [P, 2, NJ, 2, 512], F8, kind="ExternalInput").ap()
    wo_d = nc.dram_tensor("wo_p", [P, DC, NJ, 2, P], F8, kind="ExternalInput").ap()
    gw1_d = nc.dram_tensor("gw1_p", [P, 4, DC, P], FP32R, kind="ExternalInput").ap()
    gw2_d = nc.dram_tensor("gw2_p", [P, 4, E], FP32R, kind="ExternalInput").ap()
    # rank-1 row tables bf16 (x WS): qg1[512], kg1[1024], vg1[1024]
    r1_d = nc.dram_tensor("r1t", [1, 512 + D + D], BF16, kind="ExternalInput").ap()
    gg1_d = nc.dram_tensor("gg1", [1, 512], FP32R, kind="ExternalInput").ap()
    pcs_d = nc.dram_tensor("pcs", [P, 2 * DC], FP32, kind="ExternalInput").ap()
    gb2_d = nc.dram_tensor("gb2", [E, 1], FP32, kind="ExternalInput").ap()

    x1T_d = nc.dram_tensor("x1T", [P, DC, SQ], FP32, kind="ExternalOutput").ap()
    h2T_d = nc.dram_tensor("h2T", [P, DC, SQ], F8, kind="ExternalOutput").ap()
    glogT_d = nc.dram_tensor("glogT", [E, SQ], FP32, kind="ExternalOutput").ap()

    with tile.TileContext(nc) as tc:
        import contextlib
        ctx = contextlib.ExitStack()
        with ctx:
            const = ctx.enter_context(tc.tile_pool(name="const", bufs=1))
            rows = ctx.enter_context(tc.tile_pool(name="rows", bufs=2))
            bcast = ctx.enter_context(tc.tile_pool(name="bcast", bufs=2))
            tmp_f = ctx.enter_context(tc.tile_pool(name="tmpf", bufs=2))
            qt_pool = ctx.enter_context(tc.tile_pool(name="qt", bufs=4))
            ut_pool = ctx.enter_context(tc.tile_pool(name="ut", bufs=NJ))
            xin = ctx.enter_context(tc.tile_pool(name="xin", bufs=1))
            wts = ctx.enter_context(tc.tile_pool(name="wts", bufs=1))
            ps_main = ctx.enter_context(tc.tile_pool(name="psm", bufs=2, space="PSUM"))
            ps_sc = ctx.enter_context(tc.tile_pool(name="pssc", bufs=3, space="PSUM"))
            ps_att = ctx.enter_context(tc.tile_pool(name="psat", bufs=3, space="PSUM"))

            # ---- inputs: earliest consumer's bytes first --------------------
            xp = xin.tile([P, NJ, 2, S], F8, tag="xp", name="xp")
            nc.sync.dma_start(xp[:, :, :, 0:512], xp_d[:, :, :, 0:512])
            nc.sync.dma_start(xp[:, :, :, 512:1024], xp_d[:, :, :, 512:1024])
            wk8 = wts.tile([P, DC, NJ, 2, P], F8, tag="wk8", name="wk8")
            nc.scalar.dma_start(wk8[:], wk_d[:])
            wq8 = wts.tile([P, 4, NJ, 2, P], F8, tag="wq8", name="wq8")
            nc.scalar.dma_start(wq8[:], wq_d[:])
            wv8 = wts.tile([P, 2, NJ, 2, 512], F8, tag="wv8", name="wv8")
            wo8 = wts.tile([P, DC, NJ, 2, P], F8, tag="wo8", name="wo8")
            gwb = wts.tile([P, 4, DC, P], FP32R, tag="gw1", name="gw1")
            gw2b = wts.tile([P, 4, E], FP32R, tag="gw2", name="gw2")
            xq = xin.tile([P, DC, SQ], BF16, tag="xq", name="xq")

            # ---- constants -------------------------------------------------
            ones_f = const.tile([P, 1], FP32)
            nc.vector.memset(ones_f[:], 1.0)
            ones_r = const.tile([P, 1], FP32R)
            nc.scalar.copy(ones_r[:], ones_f[:])
            ones_row_f = const.tile([1, P], FP32)
            nc.vector.memset(ones_row_f[:], 1.0)
            ones_row = const.tile([1, P], FP32R)
            nc.scalar.copy(ones_row[:], ones_row_f[:])
            ones_sq_f = const.tile([P, 64], FP32)
            nc.vector.memset(ones_sq_f[:], 1.0)
            ones_sq = const.tile([P, 64], FP32R)
            nc.scalar.copy(ones_sq[:], ones_sq_f[:])
            ones16 = const.tile([1, 1], BF16)
            nc.vector.memset(ones16[:], 1.0)
            ones8_t = const.tile([P, 2, 16], F8)
            nc.vector.memset(ones8_t[:], 1.0)
            ones8 = ones8_t[:, :, 0:1]      # pair-axis step 16 (ISA: step%16==0)
            c4096 = const.tile([P, 1], FP32)
            nc.vector.memset(c4096[:], 1.0 / (WS * US))
            r1t = const.tile([1, 512 + D + D], BF16, tag="r1t", name="r1t")
            # single-partition rows issue pathologically slowly on HWDGE when
            # >2KB; split into 1KB chunks
            for r1c in range(5):
                nc.sync.dma_start(r1t[:, r1c * 512:(r1c + 1) * 512],
                                  r1_d[:, r1c * 512:(r1c + 1) * 512])
            qg1 = r1t[:, 0:512]
            kg1 = r1t[:, 512:512 + D]
            vg1 = r1t[:, 512 + D:512 + 2 * D]
            Gg1 = const.tile([1, 512], FP32R, tag="gg1", name="gg1")
            nc.sync.dma_start(Gg1[:], gg1_d[:])
            Gg1 = Gg1[:, :]
            pcs = const.tile([P, 2 * DC], FP32, tag="pcs", name="pcs")
            nc.sync.dma_start(pcs[:], pcs_d[:])
            g2_pc = pcs[:, 0:DC]
            b2_pc = pcs[:, DC:2 * DC]
            gb2_pc = const.tile([E, 1], FP32)
            nc.sync.dma_start(gb2_pc[:], gb2_d[:])
            eps_b = const.tile([1, 1], FP32)
            nc.vector.memset(eps_b[:], float(EPS))

            # PE warm-up while the xp DMA lands
            warm = const.tile([P, P], BF16)
            nc.vector.memset(warm[:], 0.0)
            psw = ps_sc.tile([P, P], FP32, tag="ps_s", name="psw")
            for i in range(30):
                nc.tensor.matmul(psw[:], warm[:], warm[:], start=True, stop=True)

            # ---- LN1 stats (window n of 512 tokens) -------------------------
            att_ctx = contextlib.ExitStack()
            sq_pool = att_ctx.enter_context(tc.tile_pool(name="sqp", bufs=1))
            sq8 = sq_pool.tile([P, NJ, 2, S], F8, tag="sq8", name="sq8")

            stats = {}

            def bcast_rows(psum_pool, tag, row, w, name):
                """[1, w] row -> [128, w] sbuf via a K=1 PE matmul + copy."""
                psb = psum_pool.tile([P, w], FP32, tag=tag, name=f"psb_{name}")
                nc.tensor.matmul(psb[:], ones_row[:], row[:],
                                 start=True, stop=True)
                out = bcast.tile([P, w], FP32, tag="a_b", name=f"bc_{name}")
                nc.vector.tensor_copy(out[:], psb[:])
                return out

            def stats_win1(n):
                w = 512
                sl = slice(n * 512, (n + 1) * 512)
                for j in range(NJ):
                    for i in range(2):
                        nc.vector.tensor_tensor(sq8[:, j, i, sl], xp[:, j, i, sl],
                                                xp[:, j, i, sl], ALU.mult)
                psx = ps_main.tile([1, 512], FP32, tag="ps", name=f"psx{n}")
                psq = ps_main.tile([1, 512], FP32, tag="ps", name=f"psq{n}")
                for j in range(NJ):
                    nc.tensor.matmul(psx[:], ones8, xp[:, j, :, sl],
                                     start=(j == 0), stop=(j == NJ - 1), perf_mode=DR)
                for j in range(NJ):
                    nc.tensor.matmul(psq[:], ones8, sq8[:, j, :, sl],
                                     start=(j == 0), stop=(j == NJ - 1), perf_mode=DR)
                mu_row = rows.tile([1, w], FP32, tag="mu", name=f"mu{n}")
                var_row = rows.tile([1, w], FP32, tag="var", name=f"var{n}")
                t_row = rows.tile([1, w], FP32, tag="t", name=f"t{n}")
                nc.vector.tensor_scalar_mul(mu_row[:], psx[:1, :], 1.0 / D)
                nc.vector.tensor_tensor(t_row[:], mu_row[:], mu_row[:], ALU.mult)
                nc.vector.scalar_tensor_tensor(var_row[:], psq[:1, :], 1.0 / D,
                                               t_row[:], ALU.mult, ALU.subtract)
                sd_row = rows.tile([1, w], FP32, tag="t", name=f"sd{n}")
                nc.scalar.activation(sd_row[:], var_row[:], AF.Sqrt, bias=eps_b[:])
                a_row = rows.tile([1, w], FP32, tag="var", name=f"a{n}")
                nc.vector.reciprocal_approx_fast(out=a_row[:], in_=sd_row[:])
                aS_row = rows.tile([1, w], FP32R, tag="as", name=f"as{n}")
                nc.vector.tensor_scalar_mul(aS_row[:], a_row[:], 1.0 / WS)
                nmu_row = rows.tile([1, w], BF16, tag="mu2", name=f"nmu{n}")
                nc.vector.tensor_scalar_mul(nmu_row[:], mu_row[:], -1.0)
                aS16_row = rows.tile([1, w], BF16, tag="as16", name=f"as16_{n}")
                nc.vector.tensor_scalar_mul(aS16_row[:], a_row[:], 1.0 / WS)
                a_bS = bcast_rows(ps_main, "ps", aS_row, w, f"a{n}")
                stats[n] = (a_bS, nmu_row, aS16_row)

            att_ctx2 = contextlib.ExitStack()
            kt_pool = att_ctx2.enter_context(tc.tile_pool(name="kt", bufs=DC))
            va_pool = att_ctx2.enter_context(tc.tile_pool(name="va", bufs=NJ))
            ex_pool = att_ctx2.enter_context(tc.tile_pool(name="ex", bufs=14))
            nrm = att_ctx2.enter_context(tc.tile_pool(name="nrm", bufs=2))

            QT = [qt_pool.tile([P, SQ], BF16, tag="QT", name=f"QT{i}") for i in range(4)]
            KTH = [kt_pool.tile([P, S], BF16, tag="KTH", name=f"KTH{i}") for i in range(DC)]
            # V pairs: [key-part, kc-pair-slice, head-slot, 64 v | 1 ones]
            V_p = [va_pool.tile([P, 2, 16, 65], F8, tag="V_p", name=f"V_p{j}")
                   for j in range(NJ)]
            UTp = [ut_pool.tile([P, 2, SQ], F8, tag="UTp", name=f"UTp{i}")
                   for i in range(NJ)]
            for j in range(NJ):
                nc.vector.memset(V_p[j][:, :, :, 64:65], 1.0)
            acol = const.tile([P, DC], FP32, tag="acol", name="acol")

            def q_proj(ms):
                a_bS, nmu_row = stats[0][0], stats[0][1]
                for m in ms:
                    psq = ps_main.tile([P, 512], FP32, tag="ps", name=f"psQ{m}")
                    for j in range(NJ):
                        nc.tensor.matmul(psq[:], wq8[:, m, j], xp[:, j, :, 0:SQ],
                                         start=(j == 0), stop=False, perf_mode=DR)
                    nc.tensor.matmul(psq[:], qg1[:, m * P:(m + 1) * P], nmu_row[:],
                                     start=False, stop=True, skip_group_check=True)
                    nc.vector.tensor_tensor(QT[m][:], psq[:], a_bS[:], ALU.mult)

            def k_proj(p, n):
                a_bS, nmu_row = stats[n][0], stats[n][1]
                sl = slice(n * 512, (n + 1) * 512)
                psk = ps_main.tile([P, 512], FP32, tag="ps", name=f"psK{p}_{n}")
                for j in range(NJ):
                    nc.tensor.matmul(psk[:], wk8[:, p, j], xp[:, j, :, sl],
                                     start=(j == 0), stop=False, perf_mode=DR)
                nc.tensor.matmul(psk[:], kg1[:, p * P:(p + 1) * P], nmu_row[:],
                                 start=False, stop=True, skip_group_check=True)
                nc.vector.tensor_tensor(KTH[p][:, sl], psk[:], a_bS[:], ALU.mult)

            def v_proj(n, scs):
                for sc in scs:
                    w = sc // 4     # token window of this block
                    nmu_row = stats[w][1]
                    psv = ps_main.tile([P, 512], FP32, tag="ps", name=f"psV{n}_{sc}")
                    for j in range(NJ):
                        nc.tensor.matmul(psv[:], xp[:, j, :, sc * P:(sc + 1) * P],
                                         wv8[:, n, j], start=(j == 0), stop=False,
                                         perf_mode=DR)
                    nc.tensor.matmul(psv[:],
                                     nmu_row[:, (sc % 4) * P:(sc % 4 + 1) * P],
                                     vg1[:, n * 512:(n + 1) * 512],
                                     start=False, stop=True, skip_group_check=True)
                    nc.vector.tensor_scalar(
                        V_p[sc // 2][:, sc % 2, n * 8:(n + 1) * 8, 0:64],
                        psv.rearrange("p (h d) -> p h d", d=64),
                        acol[:, sc:sc + 1], None, ALU.mult)

            def sc_half(p, hi, js=range(NJ)):
                off = hi * 64
                slot = 2 * p + hi
                g = SLOT_HEAD[slot] // 2
                mq, qoff = g // 2, (g % 2) * 64
                assert qoff == off
                expS = sc_half.exp.setdefault(slot, {})
                for j in js:
                    expS[j] = ex_pool.tile([P, 2, SQ], F8, tag="expS",
                                           name=f"expS{slot}_{j}")
                    for i in range(2):
                        kc = 2 * j + i
                        pss = ps_sc.tile([P, 512], FP32, tag="ps_s",
                                         name=f"s{slot}_{kc}")
                        nc.tensor.matmul(pss[:],
                                         KTH[p][off:off + 64, kc * P:(kc + 1) * P],
                                         QT[mq][qoff:qoff + 64, :],
                                         start=True, stop=True)
                        nc.scalar.activation(expS[j][:, i, :], pss[:], AF.Exp,
                                             scale=SCALE)
            sc_half.exp = {}

            def av_mms(p, hi):
                slot = 2 * p + hi
                expS = sc_half.exp.pop(slot)
                psa = ps_att.tile([65, 512], FP32, tag="pa", name=f"a{slot}")
                for j in range(NJ):
                    nc.tensor.matmul(psa[:], V_p[j][:, :, slot, :], expS[j][:],
                                     start=(j == 0), stop=(j == NJ - 1),
                                     perf_mode=DR)
                av_mms.psa[slot] = psa
            av_mms.psa = {}

            def av_norm(p):
                psas = [av_mms.psa.pop(2 * p), av_mms.psa.pop(2 * p + 1)]
                j, i = p // 2, p % 2
                den_sb = nrm.tile([65, 1024], FP32, tag="den", name=f"ds{p}")
                den0 = nrm.tile([1, 1024], FP32, tag="den0", name=f"d{p}")
                for hi in range(2):
                    nc.vector.tensor_copy(den_sb[64:65, hi * 512:(hi + 1) * 512],
                                          psas[hi][64:65, :])
                    nc.sync.dma_start(den0[:, hi * 512:(hi + 1) * 512],
                                      den_sb[64:65, hi * 512:(hi + 1) * 512])
                rec0 = nrm.tile([1, 1024], FP32, tag="rec0", name=f"r{p}")
                nc.vector.reciprocal_approx_fast(out=rec0[:], in_=den0[:])
                recb = nrm.tile([64, 1024], FP32, tag="recb", name=f"rb{p}")
                nc.gpsimd.partition_broadcast(recb[:], rec0[:])
                nc.vector.scalar_tensor_tensor(UTp[j][0:64, i, :], psas[0][0:64, :],
                                               US, recb[:, 0:512], ALU.mult, ALU.mult)
                nb = nrm.tile([64, 512], F8, tag="nb", name=f"nb{p}")
                nc.vector.scalar_tensor_tensor(nb[:], psas[1][0:64, :],
                                               US, recb[:, 512:1024], ALU.mult, ALU.mult)
                nc.sync.dma_start(UTp[j][64:128, i, :], nb[:])

            # ---- schedule --------------------------------------------------
            def acols(n):
                for sc in range(n * 4, n * 4 + 4):
                    aS16_row = stats[n][2]
                    ptp = ps_main.tile([P, 1], FP32, tag="ps", name=f"tp{sc}")
                    nc.tensor.matmul(ptp[:], aS16_row[:, (sc % 4) * P:(sc % 4 + 1) * P],
                                     ones16[0:1, :], start=True, stop=True)
                    nc.vector.tensor_copy(acol[:, sc:sc + 1], ptp[:])

            stats_win1(0)
            stats_win1(1)
            nc.scalar.dma_start(wv8[:], wv_d[:])
            acols(0)
            acols(1)
            q_proj([0])
            k_proj(0, 0)
            k_proj(0, 1)
            sc_half(0, 0, [0, 1])
            q_proj([1])
            sc_half(0, 0, [2, 3])
            q_proj([2])
            sc_half(0, 1, [0, 1])
            q_proj([3])
            sc_half(0, 1, [2, 3])
            k_proj(1, 0)
            v_proj(0, range(2))
            k_proj(1, 1)
            v_proj(0, range(2, 4))
            k_proj(2, 0)
            v_proj(0, range(4, 6))
            k_proj(2, 1)
            v_proj(0, range(6, 8))
            # fine-grained interleave: AV of pair p, scores of pair p+1 and
            # K/V fillers for later pairs share the PE queue so a dependent
            # cluster cannot stall it for long.
            for p in range(8):
                nxt = p + 1 < 8
                if nxt:
                    sc_half(p + 1, 0, [0])
                av_mms(p, 0)
                if nxt:
                    sc_half(p + 1, 0, [1])
                av_mms(p, 1)
                if nxt:
                    sc_half(p + 1, 0, [2])
                av_norm(p)
                if p == 0:
                    # late-needed weights ride the gpsimd (SWDGE) ring once
                    # the early loads are done
                    nc.gpsimd.dma_start(wo8[:], wo_d[:])
                    nc.gpsimd.dma_start(gwb[:], gw1_d[:])
                    nc.gpsimd.dma_start(gw2b[:], gw2_d[:])
                    nc.gpsimd.dma_start(xq[:], xq_d[:])
                    v_proj(1, range(2))
                if nxt:
                    sc_half(p + 1, 0, [3])
                if p == 0:
                    v_proj(1, range(2, 4))
                elif p == 1:
                    v_proj(1, range(4, 6))
                if nxt:
                    sc_half(p + 1, 1, [0])
                if p == 1:
                    v_proj(1, range(6, 8))
                if p + 3 <= 7:
                    k_proj(p + 3, 0)
                if nxt:
                    sc_half(p + 1, 1, [1])
                if p + 3 <= 7:
                    k_proj(p + 3, 1)
                if nxt:
                    sc_half(p + 1, 1, [2])
                    sc_half(p + 1, 1, [3])
            att_ctx2.close()     # free KTH/V_p/expS space for the tail
            att_ctx.close()      # free sq8

            # late pools, in space vacated by the attention working set
            x1_pool = ctx.enter_context(tc.tile_pool(name="x1", bufs=1))
            h2_pool = ctx.enter_context(tc.tile_pool(name="h2", bufs=2))
            gh_pool = ctx.enter_context(tc.tile_pool(name="gh", bufs=4))
            sq2_pool = ctx.enter_context(tc.tile_pool(name="sq2", bufs=2))

            # ---- tail: out-projection + residual, LN2, folded gate ---------
            x1T = x1_pool.tile([P, DC, SQ], FP32R, tag="x1T", name="x1T")
            h2T = h2_pool.tile([P, DC, SQ], F8, tag="h2T", name="h2T")
            GhT = [gh_pool.tile([P, SQ], FP32R, tag="GhT", name=f"GhT{i}")
                   for i in range(4)]
            glog_sb = rows.tile([E, SQ], FP32, tag="glog", name="glog")
            st2 = {}

            def out_proj(w):
                sl = slice(w * 256, (w + 1) * 256)
                for m in range(DC):
                    pso = ps_main.tile([P, 256], FP32, tag="ps", name=f"psO{w}_{m}")
                    for j in range(NJ):
                        nc.tensor.matmul(pso[:], wo8[:, m, j], UTp[j][:, :, sl],
                                         start=(j == 0), stop=(j == NJ - 1),
                                         perf_mode=DR)
                    nc.vector.scalar_tensor_tensor(x1T[:, m, sl], pso[:], c4096[:],
                                                   xq[:, m, sl], ALU.mult, ALU.add)
                nc.sync.dma_start(x1T_d[:, :, sl], x1T.bitcast(FP32)[:, :, sl])

            def stats_win2(w):
                sl = slice(w * 256, (w + 1) * 256)
                ww = 256
                psx = ps_att.tile([1, 256], FP32, tag="pa", name=f"psx2_{w}")
                psq = ps_att.tile([1, 256], FP32, tag="pa", name=f"psq2_{w}")
                for kd in range(DC):
                    sq = sq2_pool.tile([P, 256], FP32R, tag="sqt", name=f"sq2_{w}_{kd}")
                    nc.vector.tensor_tensor(sq[:], x1T[:, kd, sl], x1T[:, kd, sl],
                                            ALU.mult)
                    nc.tensor.matmul(psx[:1, :], ones_r[:], x1T[:, kd, sl],
                                     start=(kd == 0), stop=(kd == DC - 1))
                    nc.tensor.matmul(psq[:1, :], ones_r[:], sq[:],
                                     start=(kd == 0), stop=(kd == DC - 1))
                mu_row = rows.tile([1, ww], FP32, tag="mu", name=f"mu2{w}")
                var_row = rows.tile([1, ww], FP32, tag="var", name=f"var2{w}")
                t_row = rows.tile([1, ww], FP32, tag="t", name=f"t2{w}")
                nc.vector.tensor_scalar_mul(mu_row[:], psx[:1, :], 1.0 / D)
                nc.vector.tensor_tensor(t_row[:], mu_row[:], mu_row[:], ALU.mult)
                nc.vector.scalar_tensor_tensor(var_row[:], psq[:1, :], 1.0 / D,
                                               t_row[:], ALU.mult, ALU.subtract)
                sd_row = rows.tile([1, ww], FP32, tag="t", name=f"sd2{w}")
                nc.scalar.activation(sd_row[:], var_row[:], AF.Sqrt, bias=eps_b[:])
                a_row = rows.tile([1, ww], FP32, tag="var", name=f"a2{w}")
                nc.vector.reciprocal_approx_fast(out=a_row[:], in_=sd_row[:])
                a_rowr = rows.tile([1, ww], FP32R, tag="ar", name=f"a2r{w}")
                nc.vector.tensor_scalar_mul(a_rowr[:], a_row[:], 1.0)
                nmu_row = rows.tile([1, ww], FP32R, tag="mu2", name=f"nmu2{w}")
                nc.vector.tensor_scalar_mul(nmu_row[:], mu_row[:], -1.0)
                c_row = rows.tile([1, ww], FP32R, tag="c", name=f"c2{w}")
                nc.vector.tensor_tensor(c_row[:], nmu_row.bitcast(FP32)[:],
                                        a_row[:], ALU.mult)
                a_b = bcast_rows(ps_att, "pa", a_rowr, ww, f"a2_{w}")
                c_b = bcast_rows(ps_att, "pa", c_row, ww, f"c2_{w}")
                st2[w] = (a_b, nmu_row, c_b)

            def gate_win(w):
                a2_b, nmu2_row = st2[w][0], st2[w][1]
                sl = slice(w * 256, (w + 1) * 256)
                for m in range(4):
                    psg = ps_sc.tile([P, 256], FP32, tag="ps_s", name=f"psG{w}_{m}")
                    for kd in range(DC):
                        nc.tensor.matmul(psg[:], gwb[:, m, kd, :], x1T[:, kd, sl],
                                         start=(kd == 0), stop=False)
                    nc.tensor.matmul(psg[:], Gg1[:, m * P:(m + 1) * P],
                                     nmu2_row[:], start=False, stop=True)
                    pre = h2_pool.tile([P, 256], FP32, tag="pre", name=f"pre{w}_{m}")
                    nc.vector.tensor_tensor(pre[:], psg[:], a2_b[:], ALU.mult)
                    nc.vector.tensor_scalar_max(GhT[m][:, sl], pre[:], 0.0)
                psl = ps_att.tile([E, 256], FP32, tag="pa", name=f"psl{w}")
                for gt in range(4):
                    nc.tensor.matmul(psl[:], gw2b[:, gt, :], GhT[gt][:, sl],
                                     start=(gt == 0), stop=(gt == 3))
                nc.vector.tensor_scalar(glog_sb[:, sl], psl[:], gb2_pc[:], None,
                                        ALU.add)

            def h2_win(w):
                sl = slice(w * 256, (w + 1) * 256)
                a2_b, c2_b = st2[w][0], st2[w][2]
                for kd in range(DC):
                    t = tmp_f.tile([P, 256], FP32, tag="lnt2", name=f"l2_{w}_{kd}")
                    nc.vector.tensor_tensor(t[:], x1T[:, kd, sl], a2_b[:], ALU.mult)
                    nc.vector.tensor_tensor(t[:], t[:], c2_b[:], ALU.add)
                    nc.vector.tensor_scalar(h2T[:, kd, sl], t[:],
                                            g2_pc[:, kd:kd + 1],
                                            b2_pc[:, kd:kd + 1], ALU.mult, ALU.add)
                nc.sync.dma_start(h2T_d[:, :, sl], h2T[:, :, sl])

            out_proj(0)
            stats_win2(0)
            out_proj(1)
            gate_win(0)
            stats_win2(1)
            gate_win(1)
            nc.sync.dma_start(glogT_d[:], glog_sb[:])
            h2_win(0)
            h2_win(1)

    nc.compile()
    return nc


# ------------------------------------------------------------------- L2 moe --

def _windows(cap):
    ws = [512] * (cap // 512)
    if cap % 512:
        ws.append(cap % 512)
    return ws


def build_ffn(caps):
    """One FFN slot per entry in `caps` (uniform shapes across cores).
    Both layers fp8 DoubleRow; the renormalized gate weight is pre-divided
    by the w2 scale on the host so the psum drain stays a single op."""
    nc = bacc.Bacc("TRN2", target_bir_lowering=False, debug=False, num_devices=NCORES)

    ins, outs = [], []
    for si, cap in enumerate(caps):
        ins.append(dict(
            xg=nc.dram_tensor(f"xg{si}", [P, NJ, 2, cap], F8, kind="ExternalInput").ap(),
            w1=nc.dram_tensor(f"w1_{si}", [P, EC, NJ, 2, P], F8,
                              kind="ExternalInput").ap(),
            w2=nc.dram_tensor(f"w2_{si}", [P, DC, EC // 2, 2, P], F8,
                              kind="ExternalInput").ap(),
            eb1=nc.dram_tensor(f"eb1_{si}", [P, EC], FP32, kind="ExternalInput").ap(),
            eb2=nc.dram_tensor(f"eb2_{si}", [P, DC], FP32, kind="ExternalInput").ap(),
            wt=nc.dram_tensor(f"wt{si}", [1, cap], FP32, kind="ExternalInput").ap(),
        ))
        outs.append(nc.dram_tensor(f"y{si}", [P, DC, cap], BF16,
                                   kind="ExternalOutput").ap())

    with tile.TileContext(nc) as tc:
        import contextlib
        ctx = contextlib.ExitStack()
        with ctx:
            const = ctx.enter_context(tc.tile_pool(name="const", bufs=1))
            xg_pool = ctx.enter_context(tc.tile_pool(name="xg", bufs=1))
            hid_pool = ctx.enter_context(tc.tile_pool(name="hid", bufs=1))
            w_pool = ctx.enter_context(tc.tile_pool(name="wp", bufs=1))
            out_pool = ctx.enter_context(tc.tile_pool(name="out", bufs=1))
            ps = ctx.enter_context(tc.tile_pool(name="ps", bufs=6, space="PSUM"))

            # sync queue: small/early tensors in consumer order; the big w2
            # transfers are issued mid-stream (scalar queue) so they don't
            # starve the layer-1 inputs.
            # Inputs are spread over the three DMA queue rings (sync /
            # scalar HWDGE + gpsimd SWDGE) in consumer order: each ring
            # drains FIFO, so a big late-needed transfer queued ahead of an
            # early one starves the whole launch.
            tls = []
            for si, cap in enumerate(caps):
                io = ins[si]
                xgt = xg_pool.tile([P, NJ, 2, cap], F8, tag=f"xg_{si}",
                                   name=f"xg_{si}")
                w1t = w_pool.tile([P, EC, NJ, 2, P], F8, tag=f"w1_{si}",
                                  name=f"w1_{si}")
                eb1_pc = const.tile([P, EC], FP32, tag="eb1", name=f"eb1_{si}")
                nc.scalar.dma_start(eb1_pc[:], io["eb1"][:])
                eb2_pc = const.tile([P, DC], FP32, tag="eb2", name=f"eb2_{si}")
                nc.scalar.dma_start(eb2_pc[:], io["eb2"][:])
                wt_row = const.tile([1, caps[0]], FP32, tag="wtr", name=f"wtr{si}")
                nc.scalar.dma_start(wt_row[:1, :cap], io["wt"][:])
                wt_b = const.tile([P, caps[0]], FP32, tag="wtb", name=f"wtb{si}")
                nc.gpsimd.partition_broadcast(wt_b[:, :cap], wt_row[:1, :cap])
                w2t = w_pool.tile([P, DC, EC // 2, 2, P], F8, tag=f"w2_{si}",
                                  name=f"w2_{si}")
                tls.append((w1t, xgt, w2t, eb1_pc, eb2_pc, wt_b))
            nc.sync.dma_start(tls[0][1][:], ins[0]["xg"][:])
            nc.sync.dma_start(tls[0][0][:, 0:EC // 2], ins[0]["w1"][:, 0:EC // 2])
            nc.scalar.dma_start(tls[0][0][:, EC // 2:EC], ins[0]["w1"][:, EC // 2:EC])
            nc.sync.dma_start(tls[0][2][:], ins[0]["w2"][:])
            nc.gpsimd.dma_start(tls[1][2][:], ins[1]["w2"][:])

            # PE warm-up while the first DMAs land
            warm = const.tile([P, P], BF16)
            nc.vector.memset(warm[:], 0.0)
            psw = ps.tile([P, P], FP32, tag="ph", name="psw")
            for i in range(60):
                nc.tensor.matmul(psw[:], warm[:], warm[:], start=True, stop=True)

            for si, cap in enumerate(caps):
                w1t, xgt, w2t, eb1_pc, eb2_pc, wt_b = tls[si]
                io = ins[si]
                WSl = _windows(cap)
                OFF = [sum(WSl[:i]) for i in range(len(WSl))]

                hidT = hid_pool.tile([P, EC // 2, 2, cap], F8, tag=f"hidT{si}",
                                     name=f"hidT{si}")
                for wi, w in enumerate(WSl):
                    sl = slice(OFF[wi], OFF[wi] + w)
                    for ec in range(EC):
                        ph = ps.tile([P, w], FP32, tag="ph", name=f"ph{si}_{ec}_{wi}")
                        for j in range(NJ):
                            nc.tensor.matmul(ph[:], w1t[:, ec, j], xgt[:, j, :, sl],
                                             start=(j == 0), stop=(j == NJ - 1),
                                             perf_mode=DR)
                        nc.scalar.activation(hidT[:, ec // 2, ec % 2, sl], ph[:],
                                             AF.Gelu, bias=eb1_pc[:, ec:ec + 1],
                                             scale=1.0 / WS)
                        if wi == 0 and si == 0 and ec == 1:
                            # slot-1 layer-1 inputs load while slot-0 computes
                            nc.scalar.dma_start(tls[1][1][:], ins[1]["xg"][:])
                            nc.scalar.dma_start(tls[1][0][:], ins[1]["w1"][:])

                ostage = out_pool.tile([P, DC, cap], BF16, tag=f"os{si}",
                                       name=f"os{si}")
                for wi, w in enumerate(WSl):
                    sl = slice(OFF[wi], OFF[wi] + w)
                    for m in range(DC):
                        py = ps.tile([P, w], FP32, tag="ph", name=f"py{si}_{m}_{wi}")
                        for jj in range(EC // 2):
                            nc.tensor.matmul(py[:], w2t[:, m, jj], hidT[:, jj, :, sl],
                                             start=(jj == 0), stop=(jj == EC // 2 - 1),
                                             perf_mode=DR)
                        nc.vector.scalar_tensor_tensor(ostage[:, m, sl], py[:],
                                                       eb2_pc[:, m:m + 1],
                                                       wt_b[:, sl], ALU.add, ALU.mult)
                nc.sync.dma_start(outs[si][:], ostage[:])

    nc.compile()
    return nc


def _pack_slots(tok_lists, wt_lists):
    """Cut per-expert token lists into at most 8 slot-1 pieces (<= c1) and 8
    slot-2 pieces (<= c2), minimizing the uniform SPMD capacities c1 + c2."""
    loads = [len(t) for t in tok_lists]
    act = [e for e in range(len(loads)) if loads[e] > 0]

    def feas(c1, c2):
        n1 = {e: 0 for e in act}
        n2 = {e: -(-loads[e] // c2) for e in act}
        for _ in range(64):
            if sum(n1.values()) > NCORES:
                return None
            if sum(n2.values()) <= NCORES:
                return n1, n2
            def gain(e):
                rem = loads[e] - n1[e] * c1
                if rem <= 0:
                    return (-1, 0)
                new = -(-max(0, rem - c1) // c2)
                return (n2[e] - new, rem)
            e = max(act, key=gain)
            if gain(e)[0] <= 0:
                return None
            n1[e] += 1
            n2[e] = -(-max(0, loads[e] - n1[e] * c1) // c2)
        return None

    best = None
    for c1 in range(512, 3392, 32):
        if best is not None and best[0] <= c1 + 256:
            break
        for c2 in range(256, c1 + 32, 32):
            if best is not None and c1 + c2 >= best[0]:
                break
            r = feas(c1, c2)
            if r is not None:
                best = (c1 + c2, c1, c2, r[0], r[1])
    _, c1, c2, n1, n2 = best
    s1, s2 = [], []
    for e in act:
        off = 0
        for _ in range(n1[e]):
            sz = min(c1, loads[e] - off)
            s1.append((e, off, sz))
            off += sz
        rem = loads[e] - off
        if rem > 0:
            psz = -(-rem // n2[e])
            for _ in range(n2[e]):
                sz = min(psz, loads[e] - off)
                if sz > 0:
                    s2.append((e, off, sz))
                    off += sz
    assert len(s1) <= NCORES and len(s2) <= NCORES
    assignment = []
    for core in range(NCORES):
        slots = []
        for group in (s1, s2):
            if core < len(group):
                e, off, sz = group[core]
                slots.append((e, tok_lists[e][off:off + sz], wt_lists[e][off:off + sz]))
            else:
                slots.append((0, np.zeros(0, np.int64), np.zeros(0, np.float32)))
        assignment.append(slots)
    return (c1, c2), assignment


# --------------------------------------------------------------- host logic --

_CACHE = {}


def _exact_gate_rows(x, wq, bq, wk, bk, wv, bv, wo, bo, ln1g, ln1b, ln2g, ln2b,
                     gw1, gb1, gw2, gb2, toks):
    """Exact (float64, vectorized) gate logits for the given flat token ids."""
    f8 = np.float64
    out = np.zeros((len(toks), E), f8)
    wq8, wo8 = wq.astype(f8), wo.astype(f8)
    gw18, gw28 = gw1.astype(f8), gw2.astype(f8)
    byb = {}
    for i, t in enumerate(toks):
        byb.setdefault(int(t) // S, []).append((i, int(t) % S))
    for b, items in byb.items():
        idx = np.array([i for i, _ in items])
        sel = np.array([s for _, s in items])
        xb = x[b].astype(f8)
        mu = xb.mean(1, keepdims=True)
        va = xb.var(1, keepdims=True)
        h = (xb - mu) / np.sqrt(va + EPS) * ln1g + ln1b
        h32 = h.astype(np.float32)
        K = (h32 @ wk + bk).astype(f8)
        V = (h32 @ wv + bv).astype(f8)
        q = h[sel] @ wq8 + bq
        ao = np.empty((len(sel), D), f8)
        for hh in range(H):
            g = hh // 2
            sc = q[:, g * HD:(g + 1) * HD] @ K[:, hh * HD:(hh + 1) * HD].T * SCALE
            sc -= sc.max(axis=1, keepdims=True)
            p = np.exp(sc)
            p /= p.sum(axis=1, keepdims=True)
            ao[:, hh * HD:(hh + 1) * HD] = p @ V[:, hh * HD:(hh + 1) * HD]
        x1 = x[b, sel].astype(f8) + ao @ wo8 + bo
        mu2 = x1.mean(1, keepdims=True)
        va2 = x1.var(1, keepdims=True)
        h2 = (x1 - mu2) / np.sqrt(va2 + EPS) * ln2g + ln2b
        out[idx] = np.maximum(h2 @ gw18 + gb1, 0.0) @ gw28 + gb2
    return out


DEBUG_STATS = {}


def _attn_in_maps(x, wq, bq, wk, bk, wv, bv, wo, bo, ln1g, ln1b, ln2g, ln2b,
                  gw1, gb1, gw2, gb2):
    # head-pair permutations
    perm64 = np.concatenate([np.arange(h * HD, (h + 1) * HD)
                             for pr in range(8) for h in (LO[pr], HI[pr])])
    wk_pm, wv_pm, wo_pm = wk[:, perm64], wv[:, perm64], wo[perm64, :]
    bk_pm, bv_pm = bk[perm64], bv[perm64]

    pc = lambda v: v.reshape(-1, P).T            # [c*128] -> [128, c]
    pcs = np.concatenate([pc(ln2g), pc(ln2b)], axis=1)
    # LN folding: W^T h = (W*g)^T x * a + (-mu) * (W^T g) + (W^T b + bias);
    # the last (constant) term must be zero for this kernel build.
    wq_f = wq * ln1g[:, None]
    wk_f = wk_pm * ln1g[:, None]
    wv_f = wv_pm * ln1g[:, None]
    gw1_f = gw1 * ln2g[:, None]
    qg1 = ln1g @ wq
    kg1 = ln1g @ wk_pm
    vg1 = ln1g @ wv_pm
    Gg1 = ln2g @ gw1
    qbT = ln1b @ wq + bq
    kbT = ln1b @ wk_pm + bk_pm
    vbT = ln1b @ wv_pm + bv_pm
    GbT = ln2b @ gw1 + gb1
    for v in (qbT, kbT, vbT, GbT):
        assert np.abs(v).max() < 1e-12, "nonzero fused bias not supported"
    r1t = (np.concatenate([qg1, kg1, vg1])[None, :] * WS).astype(BF16_NP)

    wv_prep = (wv_f * WS).astype(F8_NP).reshape(NJ, 2, P, 2, 512)
    wv_prep = np.ascontiguousarray(wv_prep.transpose(2, 3, 0, 1, 4))
    shared = dict(
        wq_p=_pair_w(wq_f), wk_p=_pair_w(wk_f), wv_p=wv_prep, wo_p=_pair_w(wo_pm),
        gw1_p=np.ascontiguousarray(
            gw1_f.reshape(DC, P, 4, P).transpose(1, 2, 0, 3), np.float32),
        gw2_p=np.ascontiguousarray(
            gw2.reshape(4, P, E).transpose(1, 0, 2), np.float32),
        r1t=np.ascontiguousarray(r1t),
        gg1=np.ascontiguousarray(Gg1[None, :], np.float32),
        pcs=np.ascontiguousarray(pcs, np.float32),
        gb2=np.ascontiguousarray(gb2[:, None]))
    in_maps = []
    x8 = x.astype(F8_NP)                        # fp8 stream of x
    for c in range(NCORES):
        b, half = c // 2, c % 2
        xbT8 = x8[b].T
        xbT = x[b].T
        if half == 1:       # rotate so own tokens come first
            xbT8 = np.concatenate([xbT8[:, SQ:], xbT8[:, :SQ]], axis=1)
            xbT = np.concatenate([xbT[:, SQ:], xbT[:, :SQ]], axis=1)
        xp = np.ascontiguousarray(
            xbT8.reshape(NJ, 2, P, S).transpose(2, 0, 1, 3))
        xqh = (xbT[:, :SQ] + bo[:, None]).astype(BF16_NP)
        xqh = np.ascontiguousarray(xqh.reshape(DC, P, SQ).transpose(1, 0, 2))
        in_maps.append(dict(shared, xp=xp, xq=xqh))
    return in_maps


def kernel(**inputs):
    x = np.ascontiguousarray(np.asarray(inputs["x"], np.float32))
    get = lambda k: np.ascontiguousarray(np.asarray(inputs[k], np.float32))
    wq, wk, wv, wo = get("wq"), get("wk"), get("wv"), get("wo")
    bq, bk, bv, bo = get("bq"), get("bk"), get("bv"), get("bo")
    ln1g, ln1b, ln2g, ln2b = get("ln1_g"), get("ln1_b"), get("ln2_g"), get("ln2_b")
    gw1, gb1, gw2, gb2 = get("gw1"), get("gb1"), get("gw2"), get("gb2")
    ew1, eb1, eb2, ew2 = get("ew1"), get("eb1"), get("eb2"), get("ew2")

    if "attn" not in _CACHE:
        _CACHE["attn"] = build_attn()
    nc1 = _CACHE["attn"]
    in_maps = _attn_in_maps(x, wq, bq, wk, bk, wv, bv, wo, bo,
                            ln1g, ln1b, ln2g, ln2b, gw1, gb1, gw2, gb2)
    r1 = run_bass_kernel_spmd(nc1, in_maps, core_ids=list(range(NCORES)))

    x1 = np.empty((T, D), np.float32)
    h2b = np.empty((T, D), F8_NP)
    glog = np.empty((T, E), np.float32)
    for c in range(NCORES):
        b, half = c // 2, c % 2
        sl = slice(b * S + half * SQ, b * S + (half + 1) * SQ)
        x1[sl] = r1.results[c]["x1T"].transpose(2, 1, 0).reshape(SQ, D)
        h2b[sl] = r1.results[c]["h2T"].transpose(2, 1, 0).reshape(SQ, D)
        glog[sl] = r1.results[c]["glogT"].T

    # ---- routing: softmax -> top-k -> renorm, with exact rescue ------------
    gate_w = _softmax_np(glog)
    srt = np.sort(gate_w, axis=1)
    sus = np.where(srt[:, -2] - srt[:, -3] < SUS_MARGIN)[0]
    DEBUG_STATS["sus"] = len(sus)
    if len(sus):
        glog[sus] = _exact_gate_rows(
            x, wq, bq, wk, bk, wv, bv, wo, bo, ln1g, ln1b, ln2g, ln2b,
            gw1, gb1, gw2, gb2, sus).astype(np.float32)
        gate_w[sus] = _softmax_np(glog[sus])
    idx = np.argsort(-gate_w, axis=1, kind="stable")[:, :TOPK]
    top_w = np.take_along_axis(gate_w, idx, axis=1)
    ren = _softmax_np(top_w)

    tok_lists, wt_lists = [], []
    for e in range(E):
        sel0 = np.where(idx[:, 0] == e)[0]
        sel1 = np.where(idx[:, 1] == e)[0]
        tok_lists.append(np.concatenate([sel0, sel1]))
        wt_lists.append(np.concatenate([ren[sel0, 0], ren[sel1, 1]]).astype(np.float32))

    caps, assignment = _pack_slots(tok_lists, wt_lists)
    DEBUG_STATS["caps"] = caps
    if ("ffn", caps) not in _CACHE:
        _CACHE[("ffn", caps)] = build_ffn(caps)
    nc2 = _CACHE[("ffn", caps)]

    w1_blocks = {e: _pair_w(ew1[e]) for e in range(E)}
    w2_blocks = {e: _pair_w(ew2[e]) for e in range(E)}
    in_maps2 = []
    for c in range(NCORES):
        m = {}
        for si, (e, toks, wts) in enumerate(assignment[c]):
            cap = caps[si]
            xgT = np.zeros((P, NJ, 2, cap), F8_NP)
            if len(toks):
                sel = h2b[toks]                       # [n, D] fp8
                xgT[:, :, :, :len(toks)] = (
                    sel.reshape(-1, NJ, 2, P).transpose(3, 1, 2, 0))
            wt_arr = np.zeros((1, cap), np.float32)
            wt_arr[0, :len(toks)] = wts
            m[f"xg{si}"] = np.ascontiguousarray(xgT)
            m[f"w1_{si}"] = w1_blocks[e]
            m[f"w2_{si}"] = w2_blocks[e]
            m[f"eb1_{si}"] = np.ascontiguousarray(eb1[e].reshape(EC, P).T)
            m[f"eb2_{si}"] = np.ascontiguousarray(eb2[e].reshape(DC, P).T * WS)
            m[f"wt{si}"] = wt_arr / WS
        in_maps2.append(m)
    r2 = run_bass_kernel_spmd(nc2, in_maps2, core_ids=list(range(NCORES)))

    moe = np.zeros((T, D), np.float32)
    for c in range(NCORES):
        for si, (e, toks, wts) in enumerate(assignment[c]):
            if len(toks):
                y = r2.results[c][f"y{si}"]           # [128, DC, cap] bf16
                yt = y[:, :, :len(toks)].transpose(2, 1, 0).reshape(len(toks), D)
                moe[toks] += yt.astype(np.float32)

    return (x1 + moe).reshape(B, S, D).astype(np.float32)
